# revision 21
# baseline (speedup 1.0000x reference)
"""Trainium2 Bass kernel for the CIN-style layer:

    z   = einsum('btf,byf->bfty', x_0, x_k)            # pairwise outer products
    z   = z.reshape(bs, ts0, f, tsk)                   # flat reinterpretation
    out = einsum('btiy,nty->bni', z, conv_w) + conv_b  # strided conv reduction

Shapes: x_0 (32, 64, 256), x_k (32, 64, 256), conv_w (128, 64, 64),
conv_b (128,) -> out (32, 128, 256).

Math: with i = a*64 + m (a = i//64, m = i%64) and feature f = 4t + a the
reference reduces to a two-step factorization (~270 MFLOP vs 8.6 GFLOP naive):

    W2[b,n,t,a]      = sum_y x_k[b,y,4t+a] * conv_w[n,t,y]         (contract y)
    out[b,n,a*64+m]  = sum_t x_0[b,m,4t+a] * W2[b,n,t,a] + conv_b  (contract t)

Sharding: pure data parallel over batch, 4 samples per core, conv_w/conv_b
replicated (no collectives).

Shipped variant (VERSION=17 = v15 + output eighths on both rings +
3-deep transpose psum pool; all device data bf16, rel err ~5e-3 vs the
2e-2 gate; fp32 baseline v3 was 36us on this box, v15/v17 are ~23.7-24us):
  step 1 (flipped vs v3): stationary = dense conv_w pair-tile
      [K=128 (tau,y), M=128 n], moving = block-diag xk tile [128, 32] ->
      PSUM W2T [n | 8*(2t+g)+c2] (c = 2*c2+g = 4b+a), 32 matmuls writing
      contiguous 32-col slices; per-group contiguous fp32->bf16 casts.
      This kills v3's 9us W2 DRAM bounce outright.
  transpose: 8 PE transposes, each reading one stride-8 single-free-dim
      slice [n | (2t+g)] of W2T (BIR requires one free dim on the
      stationary) -> PSUM [2t+g | n] bf16, copied to SBUF on vector.
  step 2: stationary = zero-padded interleaved x0 tile [K=128 (2t+g),
      M=128 (g',m)], moving = transposed W2 [128, 128 n] -> PSUM
      [64g'+m | (c2, n)]; bias fused into the PSUM->SBUF add; output
      shipped bf16 in eighths issued pairwise on both HWDGE rings and
      upcast to fp32 on the host.
  DMA: input is one [128, 6272] bf16 tensor (xk2 | wt2 | x0p | bias),
      six 256KB-ish chunks spread over the sync/scalar HWDGE rings and
      the gpsimd software-DGE ring; step-1 groups are emitted in chunk
      arrival order (1,3,0,2). 8 bf16 warmup matmuls cover the DMA window.

Known dead ends kept for reference: v13 (matmul is_transpose with a
non-identity moving operand is NOT a fused transposed-matmul - the moving
side must be a permutation matrix, results were garbage), v14 (split-K
step 2 with partition-offset transposes broke correctness), v16 (scalar
ACTIVATE copies + table preload measured slower).

All layout work happens host-side in numpy; the device only runs
contiguous DMAs, matmuls, PE transposes, casts and copies.
"""

import numpy as np

BS, TS, F, NF = 32, 64, 256, 128
NCORES = 8
B = BS // NCORES  # 4 local batches per core

F32 = np.float32


# ---------------------------------------------------------------------------
# Host-side packing
# ---------------------------------------------------------------------------

def _pack_wt(conv_w: np.ndarray) -> np.ndarray:
    # WT[64q+y, 128p+n] = conv_w[n, 2p+q, y]
    wt = conv_w.transpose(1, 2, 0).reshape(32, 2, 64, NF)  # [p, q, y, n]
    wt = wt.transpose(1, 2, 0, 3)                          # [q, y, p, n]
    return np.ascontiguousarray(wt.reshape(128, 32 * NF), dtype=F32)


def _pack_xk(xk_shard: np.ndarray, dense=False) -> np.ndarray:
    # padded: XK[64q+y, 32p+16q'+c] = xk[b, y, 8p+4q+a] iff q'==q else 0
    # dense:  XKD[64q+y, 16p+c]     = xk[b, y, 8p+4q+a]          (c = 4b+a)
    xq = xk_shard.reshape(B, TS, 32, 2, 4)       # [b, y, p, q, a]
    src = xq.transpose(3, 1, 2, 0, 4)            # [q, y, p, b, a]
    if dense:
        return np.ascontiguousarray(src.reshape(128, 512))
    arr = np.zeros((2, TS, 32, 2, B, 4), dtype=F32)
    arr[0, :, :, 0] = src[0]
    arr[1, :, :, 1] = src[1]
    return arr.reshape(128, 32 * 32)


def _pack_x0(x0_shard: np.ndarray, dense=False) -> np.ndarray:
    # padded: X0L[64h+t, 128c2+64h'+m] = x0[b(c), m, 4t+a(c)] iff h'==h
    # dense:  X0D[64h+t, 64c2+m]       = x0[b(c), m, 4t+a(c)]   (c = 2*c2+h)
    xt = x0_shard.reshape(B, TS, TS, 4).transpose(0, 3, 2, 1)  # [b, a, t, m]
    flat = xt.reshape(16, TS, TS)                              # [c, t, m]
    if dense:
        arr = np.zeros((2, TS, 8, TS), dtype=F32)              # [h, t, c2, m]
        for h in (0, 1):
            arr[h] = flat[2 * np.arange(8) + h].transpose(1, 0, 2)
        return arr.reshape(128, 512)
    arr = np.zeros((2, TS, 8, 2, TS), dtype=F32)               # [h, t, c2, h', m]
    for h in (0, 1):
        arr[h, :, :, h, :] = flat[2 * np.arange(8) + h].transpose(1, 0, 2)
    return arr.reshape(128, 8 * 128)


def _unpack_out(out_pack: np.ndarray, out_full: np.ndarray, r: int) -> None:
    if VERSION == 13:
        # out_pack[n, 128c2+64g+m] = out[4r+b(c), n, a(c)*64+m], c = 2*c2+g
        o = out_pack.reshape(NF, 8, 2, TS)  # [n, c2, g, m]
        for c2 in range(8):
            for g in (0, 1):
                c = 2 * c2 + g
                b, a = divmod(c, 4)
                out_full[4 * r + b, :, a * TS:(a + 1) * TS] = o[:, c2, g, :]
        return
    # out_pack[64h+m, 128c2+n] = out[4r+b(c), n, a(c)*64+m], c = 2*c2+h
    o = out_pack.reshape(2, TS, 8, NF)  # [h, m, c2, n]
    for c2 in range(8):
        for h in (0, 1):
            c = 2 * c2 + h
            b, a = divmod(c, 4)
            out_full[4 * r + b, :, a * TS:(a + 1) * TS] = o[h, :, c2, :].T


# ---------------------------------------------------------------------------
# Device program
# ---------------------------------------------------------------------------

_prog_cache = {}


def _emit_body(nc, tc, pool, ps_pool, f32, in0_d, in1_d, wtr_d, out_d, w2b_d,
               version=2, stage="all"):
    # stage: "in" = input DMAs only, "s1" = through step-1 copies,
    #        "shuffle" = through the bounce, "all" = full kernel
    import concourse.mybir as mybir

    if version >= 3:
        # PE warm-up: ~3.4us of back-to-back matmuls on a zeroed bf16 tile
        # while the input DMAs stream in; gets the HAM clock gate to 2.4GHz
        # before step 1 starts.  No data deps -> scheduled first on PE.
        warm_s = pool.tile([128, 512], mybir.dt.bfloat16, tag="warm")
        nc.gpsimd.memset(warm_s[:], 0.0)
        ps_w = ps_pool.tile([128, 512], f32, tag="warm_ps")
        for _ in range(8):
            nc.tensor.matmul(ps_w[:, :], warm_s[:, 0:128], warm_s[:, :],
                             start=True, stop=True)

    dense = version in (6, 7, 8, 10)   # xk/x0 shipped dense, padded on-chip
    merged = version in (5, 7, 8, 9)   # single wtr DMA + single out DMA
    dual = version in (8, 9, 10)       # use both HWDGE rings (SP + ACT)
    eng2 = nc.scalar if dual else nc.sync

    nxk = 512 if dense else 1024   # xk cols in in0
    nx0 = 512 if dense else 1024   # x0 cols in in1
    in0_s = pool.tile([128, nxk + 1024], f32, tag="in0")
    nc.sync.dma_start(in0_s[:], in0_d.ap())
    wtr_s = []
    if merged:
        t_ = pool.tile([128, 3072], f32, tag="wtr")
        eng2.dma_start(t_[:], wtr_d.ap())
        wtr_s = [t_[:, 0:1024], t_[:, 1024:2048], t_[:, 2048:3072]]
    else:
        chunk_eng = [eng2, nc.sync, eng2]
        for chunk in range(3):
            t_ = pool.tile([128, 1024], f32, tag=f"wtr{chunk}")
            chunk_eng[chunk].dma_start(
                t_[:], wtr_d.ap()[:, 1024 * chunk:1024 * (chunk + 1)])
            wtr_s.append(t_[:])
    in1_s = pool.tile([128, nx0 + 128], f32, tag="in1")
    nc.sync.dma_start(in1_s[:], in1_d.ap())

    if dense:
        # zero-pad dense xk/x0 into block-diagonal lhsT layouts on-chip
        # (memsets + strided DVE copies are hidden under the DMA stream)
        xk_pad = pool.tile([128, 1024], f32, tag="xkpad")
        nc.gpsimd.memset(xk_pad[:], 0.0)
        x0_pad = pool.tile([128, 1024], f32, tag="x0pad")
        nc.gpsimd.memset(x0_pad[:], 0.0)
        for q in range(2):
            dst = xk_pad[64 * q:64 * (q + 1), :].rearrange(
                "p (a b) -> p a b", b=32)[:, :, 16 * q:16 * (q + 1)]
            src = in0_s[64 * q:64 * (q + 1), 0:512].rearrange(
                "p (a b) -> p a b", b=16)
            nc.vector.tensor_copy(dst, src)
        for h in range(2):
            dst = x0_pad[64 * h:64 * (h + 1), :].rearrange(
                "p (a b) -> p a b", b=128)[:, :, 64 * h:64 * (h + 1)]
            src = in1_s[64 * h:64 * (h + 1), 0:512].rearrange(
                "p (a b) -> p a b", b=64)
            nc.vector.tensor_copy(dst, src)
        xk_s = xk_pad[:, 0:1024]
        x0_s = x0_pad[:, 0:1024]
    else:
        xk_s = in0_s[:, 0:1024]
        x0_s = in1_s[:, 0:1024]
    bias_s = in1_s[:, nx0:nx0 + 128]

    def wt_cols(p):  # rhs tile [128, 128] for pair p
        if p < 8:
            return in0_s[:, nxk + 128 * p:nxk + 128 * (p + 1)]
        chunk, off = divmod(128 * (p - 8), 1024)
        return wtr_s[chunk][:, off:off + 128]

    if stage == "in":
        return

    # ---- step 1: W2 = xk . wT, contract y (K = 128 = (q, y)) ----
    # psum tile u holds passes 4u..4u+3 at col 128*(k%4)
    # ---- shuffle (q,c)-partitioned W2 -> t-partitioned via DRAM bounce ----
    # (a direct SBUF->SBUF partition-gather is impossible: the BIR verifier
    # rejects partition steps != 1 and the permutation needs >3 AP dims)
    w2_s = pool.tile([128, 1024], f32, tag="w2")
    w2r_s = pool.tile([128, 1024], f32, tag="w2r")

    def emit_pass(k, ps1):
        for j in range(4):
            p = 4 * k + j
            nc.tensor.matmul(
                ps1[32 * j:32 * (j + 1), 128 * (k % 4):128 * (k % 4 + 1)],
                xk_s[:, 32 * p:32 * (p + 1)],
                wt_cols(p),
                start=True,
                stop=True,
                tile_position=(0, 32 * j),
            )

    if version == 4:
        # per-pass copy + per-pass bounce-out (k fixed -> <=3 AP dims), so
        # all but the last bounce hides under step 1; readback in (h, k-half)
        # quarters, the first two of which also overlap step 1.
        # (measured WORSE on HW: per-DMA serialized overhead dominates)
        for u in range(2):
            ps1 = ps_pool.tile([128, 512], f32, tag="s1")
            for k in range(4 * u, 4 * u + 4):
                emit_pass(k, ps1)
                kk = 128 * (k % 4)
                nc.vector.tensor_copy(w2_s[:, 128 * k:128 * (k + 1)],
                                      ps1[:, kk:kk + 128])
                dstA = w2b_d.ap()[:, k].rearrange("h j q c2 n -> j q c2 h n")
                nc.sync.dma_start(dstA, w2_s[:, 128 * k:128 * (k + 1)])
            for h in range(2):
                kh = u
                dstB = w2r_s[64 * h + 32 * kh:64 * h + 32 * kh + 32, :]
                nc.sync.dma_start(dstB, w2b_d.ap()[h, 4 * kh:4 * (kh + 1)])
    else:
        for u in range(2):
            ps1 = ps_pool.tile([128, 512], f32, tag="s1")
            for k in range(4 * u, 4 * u + 4):
                emit_pass(k, ps1)
            nc.vector.tensor_copy(w2_s[:, 512 * u:512 * (u + 1)], ps1[:, :])
        if stage == "s1":
            return
        srcA = w2_s[:].rearrange("p (k n) -> p k n", k=8)
        dstA = w2b_d.ap().rearrange("h k j q c2 n -> j q c2 h k n")
        eng2.dma_start(dstA, srcA)
        dstB = w2r_s[:].rearrange("p (c2 n) -> p c2 n", c2=8)
        nc.sync.dma_start(dstB, w2b_d.ap())
        if stage == "shuffle":
            return

    # ---- step 2: out = x0 . W2, contract t (K = 128 = (h, t)) ----
    out_s = pool.tile([128, 1024], f32, tag="out")
    for u in range(2):
        ps2 = ps_pool.tile([128, 512], f32, tag="s2")
        for c2 in range(4 * u, 4 * u + 4):
            nc.tensor.matmul(
                ps2[:, 128 * (c2 % 4):128 * (c2 % 4 + 1)],
                x0_s[:, 128 * c2:128 * (c2 + 1)],
                w2r_s[:, 128 * c2:128 * (c2 + 1)],
                start=True,
                stop=True,
            )
        bias4 = bias_s.unsqueeze(1).broadcast_to([128, 4, 128])
        nc.vector.tensor_add(
            out_s[:, 512 * u:512 * (u + 1)].rearrange("p (f n) -> p f n", f=4),
            ps2[:, :].rearrange("p (f n) -> p f n", f=4),
            bias4,
        )
        if version >= 3 and not merged:
            (nc.sync if u == 0 else eng2).dma_start(
                out_d.ap()[:, 512 * u:512 * (u + 1)],
                out_s[:, 512 * u:512 * (u + 1)])
    if version == 2 or merged:
        eng2.dma_start(out_d.ap(), out_s[:])


def _pack_x0p(x0_shard: np.ndarray) -> np.ndarray:
    # x0p[2t+g, 128c2+64g'+m] = x0[b(c), m, 4t+a(c)] iff g==g', c = 2c2+g = 4b+a
    arr = np.zeros((TS, 2, 8, 2, TS), dtype=F32)  # [t, g, c2, g', m]
    for c in range(16):
        c2, g = divmod(c, 2)
        b, a = divmod(c, 4)
        arr[:, g, c2, g, :] = x0_shard[b, :, a::4].T  # [t, m]
    return arr.reshape(128, 1024)


def _emit_body_v11(nc, tc, pool, ps_pool, tp_pool, in_d, out_d, warm=8):
    """bf16 bounce-free pipeline.

    step 1 (flipped vs v3): stationary = dense conv_w pair-tile
      [K=128 (tau,y), M=128 n], moving = block-diag xk tile [128, 32 (tau',c)]
      -> PSUM W2T [n | 32p+16tau'+c] = [n | (t, c)], 32 matmuls, no bounce.
    transpose: 8 PE transposes of strided slices [n | (t, g)] (g = c parity,
      c = 2*c2+g) -> PSUM [2t+g | n] per c2, bf16.
    step 2: stationary = block-diag x0 tile [K=128 (2t+g), M=128 (g',m)],
      moving = transposed W2 [128, 128 n] -> PSUM [64g'+m | (c2, n)], the v3
      out_pack layout. Bias added in the PSUM->SBUF copy; output DMA'd bf16.
    """
    import concourse.mybir as mybir
    from concourse import masks

    bf16 = mybir.dt.bfloat16
    f32 = mybir.dt.float32

    # PE warm-up on a zeroed bf16 tile while input DMAs stream (p-state ramp)
    warm_s = pool.tile([128, 512], bf16, tag="warm")
    nc.gpsimd.memset(warm_s[:], 0.0)
    ps_w = ps_pool.tile([128, 512], f32, tag="warm_ps")
    for _ in range(warm):
        nc.tensor.matmul(ps_w[:, :], warm_s[:, 0:128], warm_s[:, :],
                         start=True, stop=True)

    ident = pool.tile([128, 128], bf16, tag="ident")
    masks.make_identity(nc, ident[:])

    # in_s cols: [0:1024 xk2 | 1024:5120 wt2 | 5120:6144 x0p | 6144:6272 bias]
    in_s = pool.tile([128, 6272], bf16, tag="in")
    nc.sync.dma_start(in_s[:, 0:1024], in_d.ap()[:, 0:1024])        # xk2
    nc.sync.dma_start(in_s[:, 1024:2048], in_d.ap()[:, 1024:2048])  # wt2 p0-7
    nc.sync.dma_start(in_s[:, 2048:3072], in_d.ap()[:, 2048:3072])  # wt2 p8-15
    nc.scalar.dma_start(in_s[:, 3072:4096], in_d.ap()[:, 3072:4096])  # p16-23
    nc.scalar.dma_start(in_s[:, 4096:5120], in_d.ap()[:, 4096:5120])  # p24-31
    nc.scalar.dma_start(in_s[:, 5120:6272], in_d.ap()[:, 5120:6272])  # x0p+bias

    xk2 = in_s[:, 0:1024]
    x0p = in_s[:, 5120:6144]
    bias_bf = in_s[:, 6144:6272]
    bias_s = pool.tile([128, 128], f32, tag="bias32")
    nc.gpsimd.tensor_copy(bias_s[:], bias_bf)  # upcast once for the DVE add

    # ---- step 1: W2T[n, 32p+16tau+c] -> psum [128, 1024] fp32 ----
    w2t_s = pool.tile([128, 1024], bf16, tag="w2t")
    ps1 = ps_pool.tile([128, 1024], f32, tag="s1")
    # pair groups ordered to match DMA-chunk arrival (sync: 0-7, 8-15 after
    # xk2; scalar ring delivers 16-23 earliest)
    for grp in (2, 0, 3, 1):
        for p in range(8 * grp, 8 * grp + 8):
            nc.tensor.matmul(
                ps1[:, 32 * p:32 * (p + 1)],
                in_s[:, 1024 + 128 * p:1024 + 128 * (p + 1)],
                xk2[:, 32 * p:32 * (p + 1)],
                start=True, stop=True,
            )
        dst = w2t_s[:].rearrange("p (c t) -> p t c", t=64)[
            :, 16 * grp:16 * (grp + 1), :]
        src = ps1[:, 256 * grp:256 * (grp + 1)].rearrange(
            "p (t c) -> p t c", c=16)
        nc.vector.tensor_copy(dst, src)

    # ---- transpose + step 2, pipelined per c2 ----
    w2tt_s = pool.tile([128, 1024], bf16, tag="w2tt")
    out_s = pool.tile([128, 1024], bf16, tag="out")
    ps2 = ps_pool.tile([128, 1024], f32, tag="s2")
    for c2 in range(8):
        pst = tp_pool.tile([128, 128], bf16, tag="tp")
        nc.tensor.transpose(pst[:], w2t_s[:, 128 * c2:128 * (c2 + 1)],
                            ident[:])
        if c2 % 2:
            nc.scalar.copy(w2tt_s[:, 128 * c2:128 * (c2 + 1)], pst[:])
        else:
            nc.vector.tensor_copy(w2tt_s[:, 128 * c2:128 * (c2 + 1)], pst[:])
        nc.tensor.matmul(
            ps2[:, 128 * c2:128 * (c2 + 1)],
            x0p[:, 128 * c2:128 * (c2 + 1)],
            w2tt_s[:, 128 * c2:128 * (c2 + 1)],
            start=True, stop=True,
        )
        if c2 == 3 or c2 == 7:
            u = c2 // 4
            bias4 = bias_s[:].unsqueeze(1).broadcast_to([128, 4, 128])
            nc.vector.tensor_add(
                out_s[:, 512 * u:512 * (u + 1)].rearrange(
                    "p (f n) -> p f n", f=4),
                ps2[:, 512 * u:512 * (u + 1)].rearrange(
                    "p (f n) -> p f n", f=4),
                bias4,
            )
            (nc.sync if u == 0 else nc.scalar).dma_start(
                out_d.ap()[:, 512 * u:512 * (u + 1)],
                out_s[:, 512 * u:512 * (u + 1)])


def _emit_body_v12(nc, tc, pool, ps_pool, tp_pool, in_d, out_d, warm=8):
    """v11 + reordered W2T columns (8*(2t+g)+c2) so each c2 transpose input
    is one stride-8 free dim; single contiguous fp32->bf16 cast; vector-only
    PSUM copies (no ACT table load); gpsimd as third input DMA ring."""
    import concourse.mybir as mybir
    from concourse import masks

    bf16 = mybir.dt.bfloat16
    f32 = mybir.dt.float32

    warm_s = pool.tile([128, 512], bf16, tag="warm")
    nc.gpsimd.memset(warm_s[:], 0.0)
    ps_w = ps_pool.tile([128, 512], f32, tag="warm_ps")
    for _ in range(warm):
        nc.tensor.matmul(ps_w[:, :], warm_s[:, 0:128], warm_s[:, :],
                         start=True, stop=True)

    ident = pool.tile([128, 128], bf16, tag="ident")
    masks.make_identity(nc, ident[:])

    # in_s cols: [0:1024 xk2 | 1024:5120 wt2 | 5120:6144 x0p | 6144:6272 bias]
    in_s = pool.tile([128, 6272], bf16, tag="in")
    nc.sync.dma_start(in_s[:, 0:1024], in_d.ap()[:, 0:1024])          # xk2
    nc.sync.dma_start(in_s[:, 1024:2048], in_d.ap()[:, 1024:2048])    # p0-7
    nc.sync.dma_start(in_s[:, 2048:3072], in_d.ap()[:, 2048:3072])    # p8-15
    nc.scalar.dma_start(in_s[:, 3072:4096], in_d.ap()[:, 3072:4096])  # p16-23
    nc.scalar.dma_start(in_s[:, 4096:5120], in_d.ap()[:, 4096:5120])  # p24-31
    nc.gpsimd.dma_start(in_s[:, 5120:6272], in_d.ap()[:, 5120:6272])  # x0p+b

    xk2 = in_s[:, 0:1024]
    x0p = in_s[:, 5120:6144]
    bias_bf = in_s[:, 6144:6272]
    bias_s = pool.tile([128, 128], f32, tag="bias32")
    nc.gpsimd.tensor_copy(bias_s[:], bias_bf)

    # ---- step 1: psum cols 32p+16tau+(8g+c2), contiguous writes ----
    w2t_s = pool.tile([128, 1024], bf16, tag="w2t")
    ps1 = ps_pool.tile([128, 1024], f32, tag="s1")
    for grp in (2, 0, 3, 1):
        for p in range(8 * grp, 8 * grp + 8):
            nc.tensor.matmul(
                ps1[:, 32 * p:32 * (p + 1)],
                in_s[:, 1024 + 128 * p:1024 + 128 * (p + 1)],
                xk2[:, 32 * p:32 * (p + 1)],
                start=True, stop=True,
            )
    nc.vector.tensor_copy(w2t_s[:], ps1[:])  # one contiguous cast

    # ---- transpose (stride-8 read) + step 2, pipelined per c2 ----
    w2tt_s = pool.tile([128, 1024], bf16, tag="w2tt")
    out_s = pool.tile([128, 1024], bf16, tag="out")
    ps2 = ps_pool.tile([128, 1024], bf16, tag="s2")
    w2t_v = w2t_s[:].rearrange("p (tg c2) -> p tg c2", c2=8)
    for c2 in range(8):
        pst = tp_pool.tile([128, 128], bf16, tag="tp")
        nc.tensor.transpose(pst[:], w2t_v[:, :, c2:c2 + 1], ident[:])
        if v16 and c2 % 2 == 0:
            nc.scalar.copy(w2tt_s[:, 128 * c2:128 * (c2 + 1)], pst[:])
        else:
            nc.vector.tensor_copy(w2tt_s[:, 128 * c2:128 * (c2 + 1)], pst[:])
        nc.tensor.matmul(
            ps2[:, 128 * c2:128 * (c2 + 1)],
            x0p[:, 128 * c2:128 * (c2 + 1)],
            w2tt_s[:, 128 * c2:128 * (c2 + 1)],
            start=True, stop=True,
        )
        if c2 == 3 or c2 == 7:
            u = c2 // 4
            bias4 = bias_s[:].unsqueeze(1).broadcast_to([128, 4, 128])
            nc.vector.tensor_add(
                out_s[:, 512 * u:512 * (u + 1)].rearrange(
                    "p (f n) -> p f n", f=4),
                ps2[:, 512 * u:512 * (u + 1)].rearrange(
                    "p (f n) -> p f n", f=4),
                bias4,
            )
            (nc.sync if u == 0 else nc.scalar).dma_start(
                out_d.ap()[:, 512 * u:512 * (u + 1)],
                out_s[:, 512 * u:512 * (u + 1)])


def _emit_body_v13(nc, tc, pool, ps_pool, in_d, out_d, warm=8):
    """v12 + transpose fused into step 2: matmul(is_transpose=True) loads the
    W2T stride-8 slice transposed as stationary and streams x0p as moving ->
    out [n | (c2, g, m)] with no PE transposes, no tp psum tiles, no copies.
    Casts are per-group (contiguous). Bias is per-partition (n)."""
    import concourse.mybir as mybir

    bf16 = mybir.dt.bfloat16
    f32 = mybir.dt.float32

    warm_s = pool.tile([128, 512], bf16, tag="warm")
    nc.gpsimd.memset(warm_s[:], 0.0)
    ps_w = ps_pool.tile([128, 512], f32, tag="warm_ps")
    for _ in range(warm):
        nc.tensor.matmul(ps_w[:, :], warm_s[:, 0:128], warm_s[:, :],
                         start=True, stop=True)

    # in_s cols: [0:1024 xk2 | 1024:5120 wt2 | 5120:6144 x0p | 6144:6272 bias]
    in_s = pool.tile([128, 6272], bf16, tag="in")
    nc.sync.dma_start(in_s[:, 0:1024], in_d.ap()[:, 0:1024])          # xk2
    nc.sync.dma_start(in_s[:, 1024:2048], in_d.ap()[:, 1024:2048])    # p0-7
    nc.scalar.dma_start(in_s[:, 2048:3072], in_d.ap()[:, 2048:3072])  # p8-15
    nc.scalar.dma_start(in_s[:, 3072:4096], in_d.ap()[:, 3072:4096])  # p16-23
    nc.gpsimd.dma_start(in_s[:, 4096:5120], in_d.ap()[:, 4096:5120])  # p24-31
    nc.gpsimd.dma_start(in_s[:, 5120:6272], in_d.ap()[:, 5120:6272])  # x0p+b

    xk2 = in_s[:, 0:1024]
    x0p = in_s[:, 5120:6144]
    bias_bf = in_s[:, 6144:6272]   # [n, j] = conv_b[n] (row-indexed)
    bias_s = pool.tile([128, 128], f32, tag="bias32")
    nc.gpsimd.tensor_copy(bias_s[:], bias_bf)

    # ---- step 1 + per-group contiguous casts ----
    w2t_s = pool.tile([128, 1024], bf16, tag="w2t")
    ps1 = ps_pool.tile([128, 1024], f32, tag="s1")
    # (start_pair, n_pairs) subgroups in chunk-arrival order; each is
    # followed by a contiguous cast of just its psum columns
    subgroups = ([(8, 8), (24, 8), (16, 4), (0, 8), (20, 4)] if v18
                 else [(8, 8), (24, 8), (0, 8), (16, 8)])
    for p0, np_ in subgroups:
        for p in range(p0, p0 + np_):
            nc.tensor.matmul(
                ps1[:, 32 * p:32 * (p + 1)],
                in_s[:, 1024 + 128 * p:1024 + 128 * (p + 1)],
                xk2[:, 32 * p:32 * (p + 1)],
                start=True, stop=True,
            )
        nc.vector.tensor_copy(w2t_s[:, 32 * p0:32 * (p0 + np_)],
                              ps1[:, 32 * p0:32 * (p0 + np_)])

    # ---- step 2: transposed-load W2T slice (stride 8) x moving x0p ----
    out_s = pool.tile([128, 1024], bf16, tag="out")
    ps2 = ps_pool.tile([128, 1024], bf16, tag="s2")
    w2t_v = w2t_s[:].rearrange("p (tg c2) -> p tg c2", c2=8)
    for c2 in range(8):
        nc.tensor.matmul(
            ps2[:, 128 * c2:128 * (c2 + 1)],
            w2t_v[:, :, c2:c2 + 1],
            x0p[:, 128 * c2:128 * (c2 + 1)],
            is_transpose=True,
            start=True, stop=True,
        )
        if c2 == 3 or c2 == 7:
            u = c2 // 4
            bias_b = bias_s[:, 0:1].broadcast_to([128, 512])
            nc.vector.tensor_add(
                out_s[:, 512 * u:512 * (u + 1)],
                ps2[:, 512 * u:512 * (u + 1)],
                bias_b,
            )
            (nc.sync if u == 0 else nc.scalar).dma_start(
                out_d.ap()[:, 512 * u:512 * (u + 1)],
                out_s[:, 512 * u:512 * (u + 1)])


def _emit_body_v14(nc, tc, pool, ps_pool, tp_pool, in_d, out_d, warm=8):
    """v13 DMA/cast structure + v12-style PE transposes, split-K step 2:
    each c2 accumulates two t-halves in PSUM, so half-A transposes+matmuls
    overlap the tail of the weight stream. Quarter-granularity output."""
    import concourse.mybir as mybir
    from concourse import masks

    bf16 = mybir.dt.bfloat16
    f32 = mybir.dt.float32

    warm_s = pool.tile([128, 512], bf16, tag="warm")
    nc.gpsimd.memset(warm_s[:], 0.0)
    ps_w = ps_pool.tile([128, 512], f32, tag="warm_ps")
    for _ in range(warm):
        nc.tensor.matmul(ps_w[:, :], warm_s[:, 0:128], warm_s[:, :],
                         start=True, stop=True)

    ident = pool.tile([128, 128], bf16, tag="ident")
    masks.make_identity(nc, ident[:])

    # in_s cols: [0:1024 xk2 | 1024:5120 wt2 | 5120:6144 x0p | 6144:6272 bias]
    in_s = pool.tile([128, 6272], bf16, tag="in")
    ap = in_d.ap()
    nc.sync.dma_start(in_s[:, 0:1024], ap[:, 0:1024])                # xk2
    nc.scalar.dma_start(in_s[:, 1024:2048], ap[:, 1024:2048])        # p0-7
    nc.gpsimd.dma_start(in_s[0:64, 5120:6144], ap[0:64, 5120:6144])  # x0p top
    nc.gpsimd.dma_start(in_s[:, 2048:3072], ap[:, 2048:3072])        # p8-15
    nc.sync.dma_start(in_s[:, 3072:4096], ap[:, 3072:4096])          # p16-23
    nc.scalar.dma_start(in_s[:, 4096:5120], ap[:, 4096:5120])        # p24-31
    nc.sync.dma_start(in_s[64:128, 5120:6144], ap[64:128, 5120:6144])  # x0p bot
    nc.scalar.dma_start(in_s[:, 6144:6272], ap[:, 6144:6272])        # bias

    xk2 = in_s[:, 0:1024]
    x0p = in_s[:, 5120:6144]
    bias_bf = in_s[:, 6144:6272]
    bias_s = pool.tile([128, 128], f32, tag="bias32")
    nc.gpsimd.tensor_copy(bias_s[:], bias_bf)

    w2t_s = pool.tile([128, 1024], bf16, tag="w2t")
    ps1 = ps_pool.tile([128, 1024], f32, tag="s1")
    w2tt_s = pool.tile([128, 1024], bf16, tag="w2tt")
    out_s = pool.tile([128, 1024], bf16, tag="out")
    ps2 = ps_pool.tile([128, 1024], f32, tag="s2")
    w2t_v = w2t_s[:].rearrange("p (tg c2) -> p tg c2", c2=8)

    def s1_group(grp):
        for p in range(8 * grp, 8 * grp + 8):
            nc.tensor.matmul(
                ps1[:, 32 * p:32 * (p + 1)],
                in_s[:, 1024 + 128 * p:1024 + 128 * (p + 1)],
                xk2[:, 32 * p:32 * (p + 1)],
                start=True, stop=True,
            )
        nc.vector.tensor_copy(w2t_s[:, 256 * grp:256 * (grp + 1)],
                              ps1[:, 256 * grp:256 * (grp + 1)])

    def s2_half(h):
        lo, hi = 64 * h, 64 * (h + 1)
        for c2 in range(8):
            pst = tp_pool.tile([128, 128], bf16, tag="tp")
            nc.tensor.transpose(pst[lo:hi, :], w2t_v[:, lo:hi, c2:c2 + 1],
                                ident[:])
            nc.vector.tensor_copy(w2tt_s[lo:hi, 128 * c2:128 * (c2 + 1)],
                                  pst[lo:hi, :])
            nc.tensor.matmul(
                ps2[:, 128 * c2:128 * (c2 + 1)],
                x0p[lo:hi, 128 * c2:128 * (c2 + 1)],
                w2tt_s[lo:hi, 128 * c2:128 * (c2 + 1)],
                start=(h == 0), stop=(h == 1),
            )
            if h == 1 and c2 % 2 == 1:
                u = c2 // 2
                bias4 = bias_s[:].unsqueeze(1).broadcast_to([128, 2, 128])
                nc.vector.tensor_add(
                    out_s[:, 256 * u:256 * (u + 1)].rearrange(
                        "p (f n) -> p f n", f=2),
                    ps2[:, 256 * u:256 * (u + 1)].rearrange(
                        "p (f n) -> p f n", f=2),
                    bias4,
                )
                (nc.sync if u % 2 == 0 else nc.scalar).dma_start(
                    out_d.ap()[:, 256 * u:256 * (u + 1)],
                    out_s[:, 256 * u:256 * (u + 1)])

    s1_group(0)
    s1_group(1)
    s2_half(0)
    s1_group(2)
    s1_group(3)
    s2_half(1)


def _emit_body_v15(nc, tc, pool, ps_pool, tp_pool, in_d, out_d, warm=8,
                   v16=False, v17=False, v18=False):
    """v13 DMA/cast structure + v12 transpose step 2 + quarter outputs.
    v16: scalar ACT-table preloaded during warmup, PSUM->SBUF copies
    alternate vector/scalar, deeper transpose pool."""
    import concourse.mybir as mybir
    from concourse import masks

    bf16 = mybir.dt.bfloat16
    f32 = mybir.dt.float32

    warm_s = pool.tile([128, 512], bf16, tag="warm")
    nc.gpsimd.memset(warm_s[:], 0.0)
    ps_w = ps_pool.tile([128, 512], f32, tag="warm_ps")
    for _ in range(warm):
        nc.tensor.matmul(ps_w[:, :], warm_s[:, 0:128], warm_s[:, :],
                         start=True, stop=True)

    ident = pool.tile([128, 128], bf16, tag="ident")
    masks.make_identity(nc, ident[:])
    if v16:
        # touch ACTIVATE during warmup so the 1.3us table load is off the
        # critical path when scalar copies run in the transpose phase
        nc.scalar.copy(warm_s[0:1, 0:1], warm_s[0:1, 1:2])

    in_s = pool.tile([128, 6272], bf16, tag="in")
    ap = in_d.ap()
    nc.sync.dma_start(in_s[:, 0:1024], ap[:, 0:1024])          # xk2
    nc.sync.dma_start(in_s[:, 1024:2048], ap[:, 1024:2048])    # p0-7
    nc.scalar.dma_start(in_s[:, 2048:3072], ap[:, 2048:3072])  # p8-15
    if v18:
        nc.scalar.dma_start(in_s[:, 3072:3584], ap[:, 3072:3584])  # p16-19
        nc.scalar.dma_start(in_s[:, 3584:4096], ap[:, 3584:4096])  # p20-23
    else:
        nc.scalar.dma_start(in_s[:, 3072:4096], ap[:, 3072:4096])  # p16-23
    nc.gpsimd.dma_start(in_s[:, 4096:5120], ap[:, 4096:5120])  # p24-31
    nc.gpsimd.dma_start(in_s[:, 5120:6272], ap[:, 5120:6272])  # x0p+bias

    xk2 = in_s[:, 0:1024]
    x0p = in_s[:, 5120:6144]
    bias_bf = in_s[:, 6144:6272]
    bias_s = pool.tile([128, 128], f32, tag="bias32")
    nc.gpsimd.tensor_copy(bias_s[:], bias_bf)

    w2t_s = pool.tile([128, 1024], bf16, tag="w2t")
    ps1 = ps_pool.tile([128, 1024], f32, tag="s1")
    # (start_pair, n_pairs) subgroups in chunk-arrival order; each is
    # followed by a contiguous cast of just its psum columns
    subgroups = ([(8, 8), (24, 8), (16, 4), (0, 8), (20, 4)] if v18
                 else [(8, 8), (24, 8), (0, 8), (16, 8)])
    for p0, np_ in subgroups:
        for p in range(p0, p0 + np_):
            nc.tensor.matmul(
                ps1[:, 32 * p:32 * (p + 1)],
                in_s[:, 1024 + 128 * p:1024 + 128 * (p + 1)],
                xk2[:, 32 * p:32 * (p + 1)],
                start=True, stop=True,
            )
        nc.vector.tensor_copy(w2t_s[:, 32 * p0:32 * (p0 + np_)],
                              ps1[:, 32 * p0:32 * (p0 + np_)])

    w2tt_s = pool.tile([128, 1024], bf16, tag="w2tt")
    out_s = pool.tile([128, 1024], bf16, tag="out")
    ps2 = ps_pool.tile([128, 1024], f32, tag="s2")
    w2t_v = w2t_s[:].rearrange("p (tg c2) -> p tg c2", c2=8)
    for c2 in range(8):
        pst = tp_pool.tile([128, 128], bf16, tag="tp")
        nc.tensor.transpose(pst[:], w2t_v[:, :, c2:c2 + 1], ident[:])
        if v16 and c2 % 2 == 0:
            nc.scalar.copy(w2tt_s[:, 128 * c2:128 * (c2 + 1)], pst[:])
        else:
            nc.vector.tensor_copy(w2tt_s[:, 128 * c2:128 * (c2 + 1)], pst[:])
        nc.tensor.matmul(
            ps2[:, 128 * c2:128 * (c2 + 1)],
            x0p[:, 128 * c2:128 * (c2 + 1)],
            w2tt_s[:, 128 * c2:128 * (c2 + 1)],
            start=True, stop=True,
        )
        if c2 % 2 == 1:
            u = c2 // 2
            bias4 = bias_s[:].unsqueeze(1).broadcast_to([128, 2, 128])
            nc.vector.tensor_add(
                out_s[:, 256 * u:256 * (u + 1)].rearrange(
                    "p (f n) -> p f n", f=2),
                ps2[:, 256 * u:256 * (u + 1)].rearrange(
                    "p (f n) -> p f n", f=2),
                bias4,
            )
            (nc.sync if u % 2 == 0 else nc.scalar).dma_start(
                out_d.ap()[:, 256 * u:256 * (u + 1)],
                out_s[:, 256 * u:256 * (u + 1)])


def _build_program(version=None):
    if version is None:
        version = VERSION
    if version in _prog_cache:
        return _prog_cache[version]

    from contextlib import ExitStack

    import concourse.bacc as bacc
    import concourse.mybir as mybir
    import concourse.tile as tile

    f32 = mybir.dt.float32
    nc = bacc.Bacc("TRN2", target_bir_lowering=False, debug=False)

    if version >= 11:
        bf16 = mybir.dt.bfloat16
        in_d = nc.dram_tensor("in_pack", [128, 6272], bf16, kind="ExternalInput")
        out_d = nc.dram_tensor("out_pack", [128, 1024], bf16,
                               kind="ExternalOutput")
        with tile.TileContext(nc) as tc, ExitStack() as ctx:
            pool = ctx.enter_context(tc.tile_pool(name="io", bufs=1))
            ps_pool = ctx.enter_context(
                tc.tile_pool(name="ps", bufs=1, space="PSUM"))
            tp_pool = ctx.enter_context(
                tc.tile_pool(name="tp", bufs=2 if version == 15 else 3,
                             space="PSUM"))
            if version >= 15:
                _emit_body_v15(nc, tc, pool, ps_pool, tp_pool, in_d, out_d,
                               v16=(version == 16),
                               v17=(version >= 17),
                               v18=(version >= 18))
            elif version >= 14:
                _emit_body_v14(nc, tc, pool, ps_pool, tp_pool, in_d, out_d)
            elif version >= 13:
                _emit_body_v13(nc, tc, pool, ps_pool, in_d, out_d)
            elif version >= 12:
                _emit_body_v12(nc, tc, pool, ps_pool, tp_pool, in_d, out_d)
            else:
                _emit_body_v11(nc, tc, pool, ps_pool, tp_pool, in_d, out_d)
        nc.compile()
        _prog_cache[version] = nc
        return nc

    dense = version in (6, 7, 8, 10)
    nx = 512 if dense else 1024
    # in0 = [xk_pack | wt chunk0 (1024)], in1 = [x0_pack | bias (128)]
    in0_d = nc.dram_tensor("in0_pack", [128, nx + 1024], f32, kind="ExternalInput")
    in1_d = nc.dram_tensor("in1_pack", [128, nx + 128], f32, kind="ExternalInput")
    wtr_d = nc.dram_tensor("wtr_pack", [128, 3072], f32, kind="ExternalInput")
    out_d = nc.dram_tensor("out_pack", [128, 1024], f32, kind="ExternalOutput")
    # bounce layout [h, k, j, q, c2, n]
    w2b_d = nc.dram_tensor("w2_bounce", [2, 8, 4, 2, 8, 128], f32)

    with tile.TileContext(nc) as tc, ExitStack() as ctx:
        pool = ctx.enter_context(tc.tile_pool(name="io", bufs=1))
        ps_pool = ctx.enter_context(tc.tile_pool(name="ps", bufs=2, space="PSUM"))
        _emit_body(nc, tc, pool, ps_pool, f32, in0_d, in1_d, wtr_d, out_d, w2b_d,
                   version=version)

    nc.compile()
    _prog_cache[version] = nc
    return nc


def pack_core_inputs(x_0, x_k, conv_w, conv_b, version=None):
    """Returns (in_maps list of 8 dicts) for run_bass_kernel_spmd."""
    if version is None:
        version = VERSION
    if version >= 11:
        import ml_dtypes
        BF = ml_dtypes.bfloat16
        wt = _pack_wt(np.asarray(conv_w, dtype=F32))
        bias = np.broadcast_to(np.asarray(conv_b, dtype=F32), (128, 128))
        x0 = np.asarray(x_0, dtype=F32)
        xk = np.asarray(x_k, dtype=F32)
        in_maps = []
        for r in range(NCORES):
            xk2 = _pack_xk(xk[B * r:B * (r + 1)])
            if version >= 12:
                # block col order (tau, g, c2): newpos 8g+c2 <- c = 2c2+g
                cperm = np.array([2 * (i % 8) + i // 8 for i in range(16)])
                xk2 = xk2.reshape(128, 32, 2, 16)[:, :, :, cperm].reshape(
                    128, 1024)
                x0l = _pack_x0p(x0[B * r:B * (r + 1)])
            else:
                x0l = _pack_x0(x0[B * r:B * (r + 1)])
            b_blk = (np.broadcast_to(
                np.asarray(conv_b, dtype=F32)[:, None], (128, 128))
                if version == 13 else bias)
            in_pack = np.concatenate([xk2, wt, x0l, b_blk], axis=1)
            in_maps.append({"in_pack": np.ascontiguousarray(
                in_pack.astype(BF))})
        return in_maps
    dense = version in (6, 7, 8, 10)
    wt = _pack_wt(np.asarray(conv_w, dtype=F32))
    bias = np.ascontiguousarray(
        np.broadcast_to(np.asarray(conv_b, dtype=F32), (128, 128))
    )
    x0 = np.asarray(x_0, dtype=F32)
    xk = np.asarray(x_k, dtype=F32)
    wtr = np.ascontiguousarray(wt[:, 1024:])  # pairs 8..31, shared by all cores
    in_maps = []
    for r in range(NCORES):
        in0 = np.concatenate(
            [_pack_xk(xk[B * r:B * (r + 1)], dense), wt[:, :1024]], axis=1)
        in1 = np.concatenate(
            [_pack_x0(x0[B * r:B * (r + 1)], dense), bias], axis=1)
        in_maps.append({
            "in0_pack": np.ascontiguousarray(in0),
            "in1_pack": np.ascontiguousarray(in1),
            "wtr_pack": wtr,
        })
    return in_maps


VERSION = 18  # current best variant


def kernel(x_0, x_k, conv_w, conv_b):
    from concourse.bass_utils import run_bass_kernel_spmd

    nc = _build_program(VERSION)
    in_maps = pack_core_inputs(x_0, x_k, conv_w, conv_b, version=VERSION)
    res = run_bass_kernel_spmd(nc, in_maps, core_ids=list(range(NCORES)))
    out = np.empty((BS, NF, F), dtype=F32)
    for r in range(NCORES):
        _unpack_out(np.asarray(res.results[r]["out_pack"], dtype=F32), out, r)
    return out


# ---------------------------------------------------------------------------
# numpy model of the packed device program (for testing the packing logic)
# ---------------------------------------------------------------------------

def _numpy_model(x_0, x_k, conv_w, conv_b):
    out = np.empty((BS, NF, F), dtype=F32)
    in_maps = pack_core_inputs(x_0, x_k, conv_w, conv_b, version=2)
    for r in range(NCORES):
        m = in_maps[r]
        xk_s = m["in0_pack"][:, :1024]
        wt = np.concatenate([m["in0_pack"][:, 1024:], m["wtr_pack"]], axis=1)
        x0l = m["in1_pack"][:, :1024]
        bias = m["in1_pack"][:, 1024:1152]
        w2 = np.zeros((128, 1024), dtype=F32)
        for k in range(8):
            ps1 = np.zeros((128, 128), dtype=F32)
            for j in range(4):
                p = 4 * k + j
                ps1[32 * j:32 * (j + 1), :] = (
                    xk_s[:, 32 * p:32 * (p + 1)].T @ wt[:, 128 * p:128 * (p + 1)]
                )
            w2[:, 128 * k:128 * (k + 1)] = ps1
        # bounce: src partition (j,q,c2,h), free (k,n) -> dst [h,k,j,q,c2,n]
        srcA = w2.reshape(4, 2, 8, 2, 8, 128)          # [j,q,c2,h,k,n]
        w2b = srcA.transpose(3, 4, 0, 1, 2, 5)         # [h,k,j,q,c2,n]
        w2r = w2b.reshape(128, 8, 128).reshape(128, 1024)  # partition (h,k,j,q)
        out_pack = np.empty((128, 1024), dtype=F32)
        for c2 in range(8):
            out_pack[:, 128 * c2:128 * (c2 + 1)] = (
                x0l[:, 128 * c2:128 * (c2 + 1)].T @ w2r[:, 128 * c2:128 * (c2 + 1)]
                + bias
            )
        _unpack_out(out_pack, out, r)
    return out



# revision 23
# speedup vs baseline: 1.0248x; 1.0248x over previous
"""Trainium2 Bass kernel for the CIN-style layer:

    z   = einsum('btf,byf->bfty', x_0, x_k)            # pairwise outer products
    z   = z.reshape(bs, ts0, f, tsk)                   # flat reinterpretation
    out = einsum('btiy,nty->bni', z, conv_w) + conv_b  # strided conv reduction

Shapes: x_0 (32, 64, 256), x_k (32, 64, 256), conv_w (128, 64, 64),
conv_b (128,) -> out (32, 128, 256).

Math: with i = a*64 + m (a = i//64, m = i%64) and feature f = 4t + a the
reference reduces to a two-step factorization (~270 MFLOP vs 8.6 GFLOP naive):

    W2[b,n,t,a]      = sum_y x_k[b,y,4t+a] * conv_w[n,t,y]         (contract y)
    out[b,n,a*64+m]  = sum_t x_0[b,m,4t+a] * W2[b,n,t,a] + conv_b  (contract t)

Sharding: pure data parallel over batch, 4 samples per core, conv_w/conv_b
replicated (no collectives).

Shipped variant (VERSION=17 = v15 + output eighths on both rings +
3-deep transpose psum pool; all device data bf16, rel err ~5e-3 vs the
2e-2 gate; fp32 baseline v3 was 36us on this box, v15/v17 are ~23.7-24us):
  step 1 (flipped vs v3): stationary = dense conv_w pair-tile
      [K=128 (tau,y), M=128 n], moving = block-diag xk tile [128, 32] ->
      PSUM W2T [n | 8*(2t+g)+c2] (c = 2*c2+g = 4b+a), 32 matmuls writing
      contiguous 32-col slices; per-group contiguous fp32->bf16 casts.
      This kills v3's 9us W2 DRAM bounce outright.
  transpose: 8 PE transposes, each reading one stride-8 single-free-dim
      slice [n | (2t+g)] of W2T (BIR requires one free dim on the
      stationary) -> PSUM [2t+g | n] bf16, copied to SBUF on vector.
  step 2: stationary = zero-padded interleaved x0 tile [K=128 (2t+g),
      M=128 (g',m)], moving = transposed W2 [128, 128 n] -> PSUM
      [64g'+m | (c2, n)]; bias fused into the PSUM->SBUF add; output
      shipped bf16 in eighths issued pairwise on both HWDGE rings and
      upcast to fp32 on the host.
  DMA: input is one [128, 6272] bf16 tensor (xk2 | wt2 | x0p | bias),
      six 256KB-ish chunks spread over the sync/scalar HWDGE rings and
      the gpsimd software-DGE ring; step-1 groups are emitted in chunk
      arrival order (1,3,0,2). 8 bf16 warmup matmuls cover the DMA window.

Known dead ends kept for reference: v13 (matmul is_transpose with a
non-identity moving operand is NOT a fused transposed-matmul - the moving
side must be a permutation matrix, results were garbage), v14 (split-K
step 2 with partition-offset transposes broke correctness), v16 (scalar
ACTIVATE copies + table preload measured slower - the preload was emitted
before scalar's DMA triggers and delayed the whole ring), v18 (splitting
the last weight chunk + its cast into halves measured slower).

All layout work happens host-side in numpy; the device only runs
contiguous DMAs, matmuls, PE transposes, casts and copies.
"""

import numpy as np

BS, TS, F, NF = 32, 64, 256, 128
NCORES = 8
B = BS // NCORES  # 4 local batches per core

F32 = np.float32


# ---------------------------------------------------------------------------
# Host-side packing
# ---------------------------------------------------------------------------

def _pack_wt(conv_w: np.ndarray) -> np.ndarray:
    # WT[64q+y, 128p+n] = conv_w[n, 2p+q, y]
    wt = conv_w.transpose(1, 2, 0).reshape(32, 2, 64, NF)  # [p, q, y, n]
    wt = wt.transpose(1, 2, 0, 3)                          # [q, y, p, n]
    return np.ascontiguousarray(wt.reshape(128, 32 * NF), dtype=F32)


def _pack_xk(xk_shard: np.ndarray, dense=False) -> np.ndarray:
    # padded: XK[64q+y, 32p+16q'+c] = xk[b, y, 8p+4q+a] iff q'==q else 0
    # dense:  XKD[64q+y, 16p+c]     = xk[b, y, 8p+4q+a]          (c = 4b+a)
    xq = xk_shard.reshape(B, TS, 32, 2, 4)       # [b, y, p, q, a]
    src = xq.transpose(3, 1, 2, 0, 4)            # [q, y, p, b, a]
    if dense:
        return np.ascontiguousarray(src.reshape(128, 512))
    arr = np.zeros((2, TS, 32, 2, B, 4), dtype=F32)
    arr[0, :, :, 0] = src[0]
    arr[1, :, :, 1] = src[1]
    return arr.reshape(128, 32 * 32)


def _pack_x0(x0_shard: np.ndarray, dense=False) -> np.ndarray:
    # padded: X0L[64h+t, 128c2+64h'+m] = x0[b(c), m, 4t+a(c)] iff h'==h
    # dense:  X0D[64h+t, 64c2+m]       = x0[b(c), m, 4t+a(c)]   (c = 2*c2+h)
    xt = x0_shard.reshape(B, TS, TS, 4).transpose(0, 3, 2, 1)  # [b, a, t, m]
    flat = xt.reshape(16, TS, TS)                              # [c, t, m]
    if dense:
        arr = np.zeros((2, TS, 8, TS), dtype=F32)              # [h, t, c2, m]
        for h in (0, 1):
            arr[h] = flat[2 * np.arange(8) + h].transpose(1, 0, 2)
        return arr.reshape(128, 512)
    arr = np.zeros((2, TS, 8, 2, TS), dtype=F32)               # [h, t, c2, h', m]
    for h in (0, 1):
        arr[h, :, :, h, :] = flat[2 * np.arange(8) + h].transpose(1, 0, 2)
    return arr.reshape(128, 8 * 128)


def _unpack_out(out_pack: np.ndarray, out_full: np.ndarray, r: int) -> None:
    if VERSION == 13:
        # out_pack[n, 128c2+64g+m] = out[4r+b(c), n, a(c)*64+m], c = 2*c2+g
        o = out_pack.reshape(NF, 8, 2, TS)  # [n, c2, g, m]
        for c2 in range(8):
            for g in (0, 1):
                c = 2 * c2 + g
                b, a = divmod(c, 4)
                out_full[4 * r + b, :, a * TS:(a + 1) * TS] = o[:, c2, g, :]
        return
    # out_pack[64h+m, 128c2+n] = out[4r+b(c), n, a(c)*64+m], c = 2*c2+h
    o = out_pack.reshape(2, TS, 8, NF)  # [h, m, c2, n]
    for c2 in range(8):
        for h in (0, 1):
            c = 2 * c2 + h
            b, a = divmod(c, 4)
            out_full[4 * r + b, :, a * TS:(a + 1) * TS] = o[h, :, c2, :].T


# ---------------------------------------------------------------------------
# Device program
# ---------------------------------------------------------------------------

_prog_cache = {}


def _emit_body(nc, tc, pool, ps_pool, f32, in0_d, in1_d, wtr_d, out_d, w2b_d,
               version=2, stage="all"):
    # stage: "in" = input DMAs only, "s1" = through step-1 copies,
    #        "shuffle" = through the bounce, "all" = full kernel
    import concourse.mybir as mybir

    if version >= 3:
        # PE warm-up: ~3.4us of back-to-back matmuls on a zeroed bf16 tile
        # while the input DMAs stream in; gets the HAM clock gate to 2.4GHz
        # before step 1 starts.  No data deps -> scheduled first on PE.
        warm_s = pool.tile([128, 512], mybir.dt.bfloat16, tag="warm")
        nc.gpsimd.memset(warm_s[:], 0.0)
        ps_w = ps_pool.tile([128, 512], f32, tag="warm_ps")
        for _ in range(8):
            nc.tensor.matmul(ps_w[:, :], warm_s[:, 0:128], warm_s[:, :],
                             start=True, stop=True)

    dense = version in (6, 7, 8, 10)   # xk/x0 shipped dense, padded on-chip
    merged = version in (5, 7, 8, 9)   # single wtr DMA + single out DMA
    dual = version in (8, 9, 10)       # use both HWDGE rings (SP + ACT)
    eng2 = nc.scalar if dual else nc.sync

    nxk = 512 if dense else 1024   # xk cols in in0
    nx0 = 512 if dense else 1024   # x0 cols in in1
    in0_s = pool.tile([128, nxk + 1024], f32, tag="in0")
    nc.sync.dma_start(in0_s[:], in0_d.ap())
    wtr_s = []
    if merged:
        t_ = pool.tile([128, 3072], f32, tag="wtr")
        eng2.dma_start(t_[:], wtr_d.ap())
        wtr_s = [t_[:, 0:1024], t_[:, 1024:2048], t_[:, 2048:3072]]
    else:
        chunk_eng = [eng2, nc.sync, eng2]
        for chunk in range(3):
            t_ = pool.tile([128, 1024], f32, tag=f"wtr{chunk}")
            chunk_eng[chunk].dma_start(
                t_[:], wtr_d.ap()[:, 1024 * chunk:1024 * (chunk + 1)])
            wtr_s.append(t_[:])
    in1_s = pool.tile([128, nx0 + 128], f32, tag="in1")
    nc.sync.dma_start(in1_s[:], in1_d.ap())

    if dense:
        # zero-pad dense xk/x0 into block-diagonal lhsT layouts on-chip
        # (memsets + strided DVE copies are hidden under the DMA stream)
        xk_pad = pool.tile([128, 1024], f32, tag="xkpad")
        nc.gpsimd.memset(xk_pad[:], 0.0)
        x0_pad = pool.tile([128, 1024], f32, tag="x0pad")
        nc.gpsimd.memset(x0_pad[:], 0.0)
        for q in range(2):
            dst = xk_pad[64 * q:64 * (q + 1), :].rearrange(
                "p (a b) -> p a b", b=32)[:, :, 16 * q:16 * (q + 1)]
            src = in0_s[64 * q:64 * (q + 1), 0:512].rearrange(
                "p (a b) -> p a b", b=16)
            nc.vector.tensor_copy(dst, src)
        for h in range(2):
            dst = x0_pad[64 * h:64 * (h + 1), :].rearrange(
                "p (a b) -> p a b", b=128)[:, :, 64 * h:64 * (h + 1)]
            src = in1_s[64 * h:64 * (h + 1), 0:512].rearrange(
                "p (a b) -> p a b", b=64)
            nc.vector.tensor_copy(dst, src)
        xk_s = xk_pad[:, 0:1024]
        x0_s = x0_pad[:, 0:1024]
    else:
        xk_s = in0_s[:, 0:1024]
        x0_s = in1_s[:, 0:1024]
    bias_s = in1_s[:, nx0:nx0 + 128]

    def wt_cols(p):  # rhs tile [128, 128] for pair p
        if p < 8:
            return in0_s[:, nxk + 128 * p:nxk + 128 * (p + 1)]
        chunk, off = divmod(128 * (p - 8), 1024)
        return wtr_s[chunk][:, off:off + 128]

    if stage == "in":
        return

    # ---- step 1: W2 = xk . wT, contract y (K = 128 = (q, y)) ----
    # psum tile u holds passes 4u..4u+3 at col 128*(k%4)
    # ---- shuffle (q,c)-partitioned W2 -> t-partitioned via DRAM bounce ----
    # (a direct SBUF->SBUF partition-gather is impossible: the BIR verifier
    # rejects partition steps != 1 and the permutation needs >3 AP dims)
    w2_s = pool.tile([128, 1024], f32, tag="w2")
    w2r_s = pool.tile([128, 1024], f32, tag="w2r")

    def emit_pass(k, ps1):
        for j in range(4):
            p = 4 * k + j
            nc.tensor.matmul(
                ps1[32 * j:32 * (j + 1), 128 * (k % 4):128 * (k % 4 + 1)],
                xk_s[:, 32 * p:32 * (p + 1)],
                wt_cols(p),
                start=True,
                stop=True,
                tile_position=(0, 32 * j),
            )

    if version == 4:
        # per-pass copy + per-pass bounce-out (k fixed -> <=3 AP dims), so
        # all but the last bounce hides under step 1; readback in (h, k-half)
        # quarters, the first two of which also overlap step 1.
        # (measured WORSE on HW: per-DMA serialized overhead dominates)
        for u in range(2):
            ps1 = ps_pool.tile([128, 512], f32, tag="s1")
            for k in range(4 * u, 4 * u + 4):
                emit_pass(k, ps1)
                kk = 128 * (k % 4)
                nc.vector.tensor_copy(w2_s[:, 128 * k:128 * (k + 1)],
                                      ps1[:, kk:kk + 128])
                dstA = w2b_d.ap()[:, k].rearrange("h j q c2 n -> j q c2 h n")
                nc.sync.dma_start(dstA, w2_s[:, 128 * k:128 * (k + 1)])
            for h in range(2):
                kh = u
                dstB = w2r_s[64 * h + 32 * kh:64 * h + 32 * kh + 32, :]
                nc.sync.dma_start(dstB, w2b_d.ap()[h, 4 * kh:4 * (kh + 1)])
    else:
        for u in range(2):
            ps1 = ps_pool.tile([128, 512], f32, tag="s1")
            for k in range(4 * u, 4 * u + 4):
                emit_pass(k, ps1)
            nc.vector.tensor_copy(w2_s[:, 512 * u:512 * (u + 1)], ps1[:, :])
        if stage == "s1":
            return
        srcA = w2_s[:].rearrange("p (k n) -> p k n", k=8)
        dstA = w2b_d.ap().rearrange("h k j q c2 n -> j q c2 h k n")
        eng2.dma_start(dstA, srcA)
        dstB = w2r_s[:].rearrange("p (c2 n) -> p c2 n", c2=8)
        nc.sync.dma_start(dstB, w2b_d.ap())
        if stage == "shuffle":
            return

    # ---- step 2: out = x0 . W2, contract t (K = 128 = (h, t)) ----
    out_s = pool.tile([128, 1024], f32, tag="out")
    for u in range(2):
        ps2 = ps_pool.tile([128, 512], f32, tag="s2")
        for c2 in range(4 * u, 4 * u + 4):
            nc.tensor.matmul(
                ps2[:, 128 * (c2 % 4):128 * (c2 % 4 + 1)],
                x0_s[:, 128 * c2:128 * (c2 + 1)],
                w2r_s[:, 128 * c2:128 * (c2 + 1)],
                start=True,
                stop=True,
            )
        bias4 = bias_s.unsqueeze(1).broadcast_to([128, 4, 128])
        nc.vector.tensor_add(
            out_s[:, 512 * u:512 * (u + 1)].rearrange("p (f n) -> p f n", f=4),
            ps2[:, :].rearrange("p (f n) -> p f n", f=4),
            bias4,
        )
        if version >= 3 and not merged:
            (nc.sync if u == 0 else eng2).dma_start(
                out_d.ap()[:, 512 * u:512 * (u + 1)],
                out_s[:, 512 * u:512 * (u + 1)])
    if version == 2 or merged:
        eng2.dma_start(out_d.ap(), out_s[:])


def _pack_x0p(x0_shard: np.ndarray) -> np.ndarray:
    # x0p[2t+g, 128c2+64g'+m] = x0[b(c), m, 4t+a(c)] iff g==g', c = 2c2+g = 4b+a
    arr = np.zeros((TS, 2, 8, 2, TS), dtype=F32)  # [t, g, c2, g', m]
    for c in range(16):
        c2, g = divmod(c, 2)
        b, a = divmod(c, 4)
        arr[:, g, c2, g, :] = x0_shard[b, :, a::4].T  # [t, m]
    return arr.reshape(128, 1024)


def _emit_body_v11(nc, tc, pool, ps_pool, tp_pool, in_d, out_d, warm=8):
    """bf16 bounce-free pipeline.

    step 1 (flipped vs v3): stationary = dense conv_w pair-tile
      [K=128 (tau,y), M=128 n], moving = block-diag xk tile [128, 32 (tau',c)]
      -> PSUM W2T [n | 32p+16tau'+c] = [n | (t, c)], 32 matmuls, no bounce.
    transpose: 8 PE transposes of strided slices [n | (t, g)] (g = c parity,
      c = 2*c2+g) -> PSUM [2t+g | n] per c2, bf16.
    step 2: stationary = block-diag x0 tile [K=128 (2t+g), M=128 (g',m)],
      moving = transposed W2 [128, 128 n] -> PSUM [64g'+m | (c2, n)], the v3
      out_pack layout. Bias added in the PSUM->SBUF copy; output DMA'd bf16.
    """
    import concourse.mybir as mybir
    from concourse import masks

    bf16 = mybir.dt.bfloat16
    f32 = mybir.dt.float32

    # PE warm-up on a zeroed bf16 tile while input DMAs stream (p-state ramp)
    warm_s = pool.tile([128, 512], bf16, tag="warm")
    nc.gpsimd.memset(warm_s[:], 0.0)
    ps_w = ps_pool.tile([128, 512], f32, tag="warm_ps")
    for _ in range(warm):
        nc.tensor.matmul(ps_w[:, :], warm_s[:, 0:128], warm_s[:, :],
                         start=True, stop=True)

    ident = pool.tile([128, 128], bf16, tag="ident")
    masks.make_identity(nc, ident[:])

    # in_s cols: [0:1024 xk2 | 1024:5120 wt2 | 5120:6144 x0p | 6144:6272 bias]
    in_s = pool.tile([128, 6272], bf16, tag="in")
    nc.sync.dma_start(in_s[:, 0:1024], in_d.ap()[:, 0:1024])        # xk2
    nc.sync.dma_start(in_s[:, 1024:2048], in_d.ap()[:, 1024:2048])  # wt2 p0-7
    nc.sync.dma_start(in_s[:, 2048:3072], in_d.ap()[:, 2048:3072])  # wt2 p8-15
    nc.scalar.dma_start(in_s[:, 3072:4096], in_d.ap()[:, 3072:4096])  # p16-23
    nc.scalar.dma_start(in_s[:, 4096:5120], in_d.ap()[:, 4096:5120])  # p24-31
    nc.scalar.dma_start(in_s[:, 5120:6272], in_d.ap()[:, 5120:6272])  # x0p+bias

    xk2 = in_s[:, 0:1024]
    x0p = in_s[:, 5120:6144]
    bias_bf = in_s[:, 6144:6272]
    if v19:
        ones_s = pool.tile([1, 128], bf16, tag="ones")
        nc.gpsimd.memset(ones_s[:], 1.0)
    else:
        bias_s = pool.tile([128, 128], f32, tag="bias32")
        nc.gpsimd.tensor_copy(bias_s[:], bias_bf)  # upcast once for the DVE add

    # ---- step 1: W2T[n, 32p+16tau+c] -> psum [128, 1024] fp32 ----
    w2t_s = pool.tile([128, 1024], bf16, tag="w2t")
    ps1 = ps_pool.tile([128, 1024], f32, tag="s1")
    # pair groups ordered to match DMA-chunk arrival (sync: 0-7, 8-15 after
    # xk2; scalar ring delivers 16-23 earliest)
    for grp in (2, 0, 3, 1):
        for p in range(8 * grp, 8 * grp + 8):
            nc.tensor.matmul(
                ps1[:, 32 * p:32 * (p + 1)],
                in_s[:, 1024 + 128 * p:1024 + 128 * (p + 1)],
                xk2[:, 32 * p:32 * (p + 1)],
                start=True, stop=True,
            )
        dst = w2t_s[:].rearrange("p (c t) -> p t c", t=64)[
            :, 16 * grp:16 * (grp + 1), :]
        src = ps1[:, 256 * grp:256 * (grp + 1)].rearrange(
            "p (t c) -> p t c", c=16)
        nc.vector.tensor_copy(dst, src)

    # ---- transpose + step 2, pipelined per c2 ----
    w2tt_s = pool.tile([128, 1024], bf16, tag="w2tt")
    out_s = pool.tile([128, 1024], bf16, tag="out")
    ps2 = ps_pool.tile([128, 1024], f32, tag="s2")
    for c2 in range(8):
        pst = tp_pool.tile([128, 128], bf16, tag="tp")
        nc.tensor.transpose(pst[:], w2t_s[:, 128 * c2:128 * (c2 + 1)],
                            ident[:])
        if c2 % 2:
            nc.scalar.copy(w2tt_s[:, 128 * c2:128 * (c2 + 1)], pst[:])
        else:
            nc.vector.tensor_copy(w2tt_s[:, 128 * c2:128 * (c2 + 1)], pst[:])
        nc.tensor.matmul(
            ps2[:, 128 * c2:128 * (c2 + 1)],
            x0p[:, 128 * c2:128 * (c2 + 1)],
            w2tt_s[:, 128 * c2:128 * (c2 + 1)],
            start=True, stop=True,
        )
        if c2 == 3 or c2 == 7:
            u = c2 // 4
            bias4 = bias_s[:].unsqueeze(1).broadcast_to([128, 4, 128])
            nc.vector.tensor_add(
                out_s[:, 512 * u:512 * (u + 1)].rearrange(
                    "p (f n) -> p f n", f=4),
                ps2[:, 512 * u:512 * (u + 1)].rearrange(
                    "p (f n) -> p f n", f=4),
                bias4,
            )
            (nc.sync if u == 0 else nc.scalar).dma_start(
                out_d.ap()[:, 512 * u:512 * (u + 1)],
                out_s[:, 512 * u:512 * (u + 1)])


def _emit_body_v12(nc, tc, pool, ps_pool, tp_pool, in_d, out_d, warm=8):
    """v11 + reordered W2T columns (8*(2t+g)+c2) so each c2 transpose input
    is one stride-8 free dim; single contiguous fp32->bf16 cast; vector-only
    PSUM copies (no ACT table load); gpsimd as third input DMA ring."""
    import concourse.mybir as mybir
    from concourse import masks

    bf16 = mybir.dt.bfloat16
    f32 = mybir.dt.float32

    warm_s = pool.tile([128, 512], bf16, tag="warm")
    nc.gpsimd.memset(warm_s[:], 0.0)
    ps_w = ps_pool.tile([128, 512], f32, tag="warm_ps")
    for _ in range(warm):
        nc.tensor.matmul(ps_w[:, :], warm_s[:, 0:128], warm_s[:, :],
                         start=True, stop=True)

    ident = pool.tile([128, 128], bf16, tag="ident")
    masks.make_identity(nc, ident[:])

    # in_s cols: [0:1024 xk2 | 1024:5120 wt2 | 5120:6144 x0p | 6144:6272 bias]
    in_s = pool.tile([128, 6272], bf16, tag="in")
    nc.sync.dma_start(in_s[:, 0:1024], in_d.ap()[:, 0:1024])          # xk2
    nc.sync.dma_start(in_s[:, 1024:2048], in_d.ap()[:, 1024:2048])    # p0-7
    nc.sync.dma_start(in_s[:, 2048:3072], in_d.ap()[:, 2048:3072])    # p8-15
    nc.scalar.dma_start(in_s[:, 3072:4096], in_d.ap()[:, 3072:4096])  # p16-23
    nc.scalar.dma_start(in_s[:, 4096:5120], in_d.ap()[:, 4096:5120])  # p24-31
    nc.gpsimd.dma_start(in_s[:, 5120:6272], in_d.ap()[:, 5120:6272])  # x0p+b

    xk2 = in_s[:, 0:1024]
    x0p = in_s[:, 5120:6144]
    bias_bf = in_s[:, 6144:6272]
    if v19:
        ones_s = pool.tile([1, 128], bf16, tag="ones")
        nc.gpsimd.memset(ones_s[:], 1.0)
    else:
        bias_s = pool.tile([128, 128], f32, tag="bias32")
        nc.gpsimd.tensor_copy(bias_s[:], bias_bf)

    # ---- step 1: psum cols 32p+16tau+(8g+c2), contiguous writes ----
    w2t_s = pool.tile([128, 1024], bf16, tag="w2t")
    ps1 = ps_pool.tile([128, 1024], f32, tag="s1")
    for grp in (2, 0, 3, 1):
        for p in range(8 * grp, 8 * grp + 8):
            nc.tensor.matmul(
                ps1[:, 32 * p:32 * (p + 1)],
                in_s[:, 1024 + 128 * p:1024 + 128 * (p + 1)],
                xk2[:, 32 * p:32 * (p + 1)],
                start=True, stop=True,
            )
    nc.vector.tensor_copy(w2t_s[:], ps1[:])  # one contiguous cast

    # ---- transpose (stride-8 read) + step 2, pipelined per c2 ----
    w2tt_s = pool.tile([128, 1024], bf16, tag="w2tt")
    out_s = pool.tile([128, 1024], bf16, tag="out")
    ps2 = ps_pool.tile([128, 1024], bf16, tag="s2")
    w2t_v = w2t_s[:].rearrange("p (tg c2) -> p tg c2", c2=8)
    for c2 in range(8):
        pst = tp_pool.tile([128, 128], bf16, tag="tp")
        nc.tensor.transpose(pst[:], w2t_v[:, :, c2:c2 + 1], ident[:])
        if v16 and c2 % 2 == 0:
            nc.scalar.copy(w2tt_s[:, 128 * c2:128 * (c2 + 1)], pst[:])
        else:
            nc.vector.tensor_copy(w2tt_s[:, 128 * c2:128 * (c2 + 1)], pst[:])
        nc.tensor.matmul(
            ps2[:, 128 * c2:128 * (c2 + 1)],
            x0p[:, 128 * c2:128 * (c2 + 1)],
            w2tt_s[:, 128 * c2:128 * (c2 + 1)],
            start=True, stop=True,
        )
        if c2 == 3 or c2 == 7:
            u = c2 // 4
            bias4 = bias_s[:].unsqueeze(1).broadcast_to([128, 4, 128])
            nc.vector.tensor_add(
                out_s[:, 512 * u:512 * (u + 1)].rearrange(
                    "p (f n) -> p f n", f=4),
                ps2[:, 512 * u:512 * (u + 1)].rearrange(
                    "p (f n) -> p f n", f=4),
                bias4,
            )
            (nc.sync if u == 0 else nc.scalar).dma_start(
                out_d.ap()[:, 512 * u:512 * (u + 1)],
                out_s[:, 512 * u:512 * (u + 1)])


def _emit_body_v13(nc, tc, pool, ps_pool, in_d, out_d, warm=8):
    """v12 + transpose fused into step 2: matmul(is_transpose=True) loads the
    W2T stride-8 slice transposed as stationary and streams x0p as moving ->
    out [n | (c2, g, m)] with no PE transposes, no tp psum tiles, no copies.
    Casts are per-group (contiguous). Bias is per-partition (n)."""
    import concourse.mybir as mybir

    bf16 = mybir.dt.bfloat16
    f32 = mybir.dt.float32

    warm_s = pool.tile([128, 512], bf16, tag="warm")
    nc.gpsimd.memset(warm_s[:], 0.0)
    ps_w = ps_pool.tile([128, 512], f32, tag="warm_ps")
    for _ in range(warm):
        nc.tensor.matmul(ps_w[:, :], warm_s[:, 0:128], warm_s[:, :],
                         start=True, stop=True)

    # in_s cols: [0:1024 xk2 | 1024:5120 wt2 | 5120:6144 x0p | 6144:6272 bias]
    in_s = pool.tile([128, 6272], bf16, tag="in")
    nc.sync.dma_start(in_s[:, 0:1024], in_d.ap()[:, 0:1024])          # xk2
    nc.sync.dma_start(in_s[:, 1024:2048], in_d.ap()[:, 1024:2048])    # p0-7
    nc.scalar.dma_start(in_s[:, 2048:3072], in_d.ap()[:, 2048:3072])  # p8-15
    nc.scalar.dma_start(in_s[:, 3072:4096], in_d.ap()[:, 3072:4096])  # p16-23
    nc.gpsimd.dma_start(in_s[:, 4096:5120], in_d.ap()[:, 4096:5120])  # p24-31
    nc.gpsimd.dma_start(in_s[:, 5120:6272], in_d.ap()[:, 5120:6272])  # x0p+b

    xk2 = in_s[:, 0:1024]
    x0p = in_s[:, 5120:6144]
    bias_bf = in_s[:, 6144:6272]   # [n, j] = conv_b[n] (row-indexed)
    bias_s = pool.tile([128, 128], f32, tag="bias32")
    nc.gpsimd.tensor_copy(bias_s[:], bias_bf)

    # ---- step 1 + per-group contiguous casts ----
    w2t_s = pool.tile([128, 1024], bf16, tag="w2t")
    ps1 = ps_pool.tile([128, 1024], f32, tag="s1")
    # (start_pair, n_pairs) subgroups in chunk-arrival order; each is
    # followed by a contiguous cast of just its psum columns
    subgroups = ([(8, 8), (24, 8), (16, 4), (0, 8), (20, 4)] if v18
                 else [(8, 8), (24, 8), (0, 8), (16, 8)])
    for p0, np_ in subgroups:
        for p in range(p0, p0 + np_):
            nc.tensor.matmul(
                ps1[:, 32 * p:32 * (p + 1)],
                in_s[:, 1024 + 128 * p:1024 + 128 * (p + 1)],
                xk2[:, 32 * p:32 * (p + 1)],
                start=True, stop=True,
            )
        nc.vector.tensor_copy(w2t_s[:, 32 * p0:32 * (p0 + np_)],
                              ps1[:, 32 * p0:32 * (p0 + np_)])

    # ---- step 2: transposed-load W2T slice (stride 8) x moving x0p ----
    out_s = pool.tile([128, 1024], bf16, tag="out")
    ps2 = ps_pool.tile([128, 1024], bf16, tag="s2")
    w2t_v = w2t_s[:].rearrange("p (tg c2) -> p tg c2", c2=8)
    for c2 in range(8):
        nc.tensor.matmul(
            ps2[:, 128 * c2:128 * (c2 + 1)],
            w2t_v[:, :, c2:c2 + 1],
            x0p[:, 128 * c2:128 * (c2 + 1)],
            is_transpose=True,
            start=True, stop=True,
        )
        if c2 == 3 or c2 == 7:
            u = c2 // 4
            bias_b = bias_s[:, 0:1].broadcast_to([128, 512])
            nc.vector.tensor_add(
                out_s[:, 512 * u:512 * (u + 1)],
                ps2[:, 512 * u:512 * (u + 1)],
                bias_b,
            )
            (nc.sync if u == 0 else nc.scalar).dma_start(
                out_d.ap()[:, 512 * u:512 * (u + 1)],
                out_s[:, 512 * u:512 * (u + 1)])


def _emit_body_v14(nc, tc, pool, ps_pool, tp_pool, in_d, out_d, warm=8):
    """v13 DMA/cast structure + v12-style PE transposes, split-K step 2:
    each c2 accumulates two t-halves in PSUM, so half-A transposes+matmuls
    overlap the tail of the weight stream. Quarter-granularity output."""
    import concourse.mybir as mybir
    from concourse import masks

    bf16 = mybir.dt.bfloat16
    f32 = mybir.dt.float32

    warm_s = pool.tile([128, 512], bf16, tag="warm")
    nc.gpsimd.memset(warm_s[:], 0.0)
    ps_w = ps_pool.tile([128, 512], f32, tag="warm_ps")
    for _ in range(warm):
        nc.tensor.matmul(ps_w[:, :], warm_s[:, 0:128], warm_s[:, :],
                         start=True, stop=True)

    ident = pool.tile([128, 128], bf16, tag="ident")
    masks.make_identity(nc, ident[:])

    # in_s cols: [0:1024 xk2 | 1024:5120 wt2 | 5120:6144 x0p | 6144:6272 bias]
    in_s = pool.tile([128, 6272], bf16, tag="in")
    ap = in_d.ap()
    nc.sync.dma_start(in_s[:, 0:1024], ap[:, 0:1024])                # xk2
    nc.scalar.dma_start(in_s[:, 1024:2048], ap[:, 1024:2048])        # p0-7
    nc.gpsimd.dma_start(in_s[0:64, 5120:6144], ap[0:64, 5120:6144])  # x0p top
    nc.gpsimd.dma_start(in_s[:, 2048:3072], ap[:, 2048:3072])        # p8-15
    nc.sync.dma_start(in_s[:, 3072:4096], ap[:, 3072:4096])          # p16-23
    nc.scalar.dma_start(in_s[:, 4096:5120], ap[:, 4096:5120])        # p24-31
    nc.sync.dma_start(in_s[64:128, 5120:6144], ap[64:128, 5120:6144])  # x0p bot
    nc.scalar.dma_start(in_s[:, 6144:6272], ap[:, 6144:6272])        # bias

    xk2 = in_s[:, 0:1024]
    x0p = in_s[:, 5120:6144]
    bias_bf = in_s[:, 6144:6272]
    if v19:
        ones_s = pool.tile([1, 128], bf16, tag="ones")
        nc.gpsimd.memset(ones_s[:], 1.0)
    else:
        bias_s = pool.tile([128, 128], f32, tag="bias32")
        nc.gpsimd.tensor_copy(bias_s[:], bias_bf)

    w2t_s = pool.tile([128, 1024], bf16, tag="w2t")
    ps1 = ps_pool.tile([128, 1024], f32, tag="s1")
    w2tt_s = pool.tile([128, 1024], bf16, tag="w2tt")
    out_s = pool.tile([128, 1024], bf16, tag="out")
    ps2 = ps_pool.tile([128, 1024], f32, tag="s2")
    w2t_v = w2t_s[:].rearrange("p (tg c2) -> p tg c2", c2=8)

    def s1_group(grp):
        for p in range(8 * grp, 8 * grp + 8):
            nc.tensor.matmul(
                ps1[:, 32 * p:32 * (p + 1)],
                in_s[:, 1024 + 128 * p:1024 + 128 * (p + 1)],
                xk2[:, 32 * p:32 * (p + 1)],
                start=True, stop=True,
            )
        nc.vector.tensor_copy(w2t_s[:, 256 * grp:256 * (grp + 1)],
                              ps1[:, 256 * grp:256 * (grp + 1)])

    def s2_half(h):
        lo, hi = 64 * h, 64 * (h + 1)
        for c2 in range(8):
            pst = tp_pool.tile([128, 128], bf16, tag="tp")
            nc.tensor.transpose(pst[lo:hi, :], w2t_v[:, lo:hi, c2:c2 + 1],
                                ident[:])
            nc.vector.tensor_copy(w2tt_s[lo:hi, 128 * c2:128 * (c2 + 1)],
                                  pst[lo:hi, :])
            nc.tensor.matmul(
                ps2[:, 128 * c2:128 * (c2 + 1)],
                x0p[lo:hi, 128 * c2:128 * (c2 + 1)],
                w2tt_s[lo:hi, 128 * c2:128 * (c2 + 1)],
                start=(h == 0), stop=(h == 1),
            )
            if h == 1 and c2 % 2 == 1:
                u = c2 // 2
                bias4 = bias_s[:].unsqueeze(1).broadcast_to([128, 2, 128])
                nc.vector.tensor_add(
                    out_s[:, 256 * u:256 * (u + 1)].rearrange(
                        "p (f n) -> p f n", f=2),
                    ps2[:, 256 * u:256 * (u + 1)].rearrange(
                        "p (f n) -> p f n", f=2),
                    bias4,
                )
                (nc.sync if u % 2 == 0 else nc.scalar).dma_start(
                    out_d.ap()[:, 256 * u:256 * (u + 1)],
                    out_s[:, 256 * u:256 * (u + 1)])

    s1_group(0)
    s1_group(1)
    s2_half(0)
    s1_group(2)
    s1_group(3)
    s2_half(1)


def _emit_body_v15(nc, tc, pool, ps_pool, tp_pool, in_d, out_d, warm=8,
                   v16=False, v17=False, v18=False, v19=False):
    """v13 DMA/cast structure + v12 transpose step 2 + quarter outputs.
    v16: scalar ACT-table preloaded during warmup, PSUM->SBUF copies
    alternate vector/scalar, deeper transpose pool."""
    import concourse.mybir as mybir
    from concourse import masks

    bf16 = mybir.dt.bfloat16
    f32 = mybir.dt.float32

    warm_s = pool.tile([128, 512], bf16, tag="warm")
    nc.gpsimd.memset(warm_s[:], 0.0)
    ps_w = ps_pool.tile([128, 512], f32, tag="warm_ps")
    for _ in range(warm):
        nc.tensor.matmul(ps_w[:, :], warm_s[:, 0:128], warm_s[:, :],
                         start=True, stop=True)

    ident = pool.tile([128, 128], bf16, tag="ident")
    masks.make_identity(nc, ident[:])
    if v16:
        # touch ACTIVATE during warmup so the 1.3us table load is off the
        # critical path when scalar copies run in the transpose phase
        nc.scalar.copy(warm_s[0:1, 0:1], warm_s[0:1, 1:2])

    in_s = pool.tile([128, 6272], bf16, tag="in")
    ap = in_d.ap()
    nc.sync.dma_start(in_s[:, 0:1024], ap[:, 0:1024])          # xk2
    nc.sync.dma_start(in_s[:, 1024:2048], ap[:, 1024:2048])    # p0-7
    nc.scalar.dma_start(in_s[:, 2048:3072], ap[:, 2048:3072])  # p8-15
    if v18:
        nc.scalar.dma_start(in_s[:, 3072:3584], ap[:, 3072:3584])  # p16-19
        nc.scalar.dma_start(in_s[:, 3584:4096], ap[:, 3584:4096])  # p20-23
    else:
        nc.scalar.dma_start(in_s[:, 3072:4096], ap[:, 3072:4096])  # p16-23
    nc.gpsimd.dma_start(in_s[:, 4096:5120], ap[:, 4096:5120])  # p24-31
    nc.gpsimd.dma_start(in_s[:, 5120:6272], ap[:, 5120:6272])  # x0p+bias

    xk2 = in_s[:, 0:1024]
    x0p = in_s[:, 5120:6144]
    bias_bf = in_s[:, 6144:6272]
    if v19:
        ones_s = pool.tile([1, 128], bf16, tag="ones")
        nc.gpsimd.memset(ones_s[:], 1.0)
    else:
        bias_s = pool.tile([128, 128], f32, tag="bias32")
        nc.gpsimd.tensor_copy(bias_s[:], bias_bf)

    w2t_s = pool.tile([128, 1024], bf16, tag="w2t")
    ps1 = ps_pool.tile([128, 1024], f32, tag="s1")
    # (start_pair, n_pairs) subgroups in chunk-arrival order; each is
    # followed by a contiguous cast of just its psum columns
    subgroups = ([(8, 8), (24, 8), (16, 4), (0, 8), (20, 4)] if v18
                 else [(8, 8), (24, 8), (0, 8), (16, 8)])
    for p0, np_ in subgroups:
        for p in range(p0, p0 + np_):
            nc.tensor.matmul(
                ps1[:, 32 * p:32 * (p + 1)],
                in_s[:, 1024 + 128 * p:1024 + 128 * (p + 1)],
                xk2[:, 32 * p:32 * (p + 1)],
                start=True, stop=True,
            )
        nc.vector.tensor_copy(w2t_s[:, 32 * p0:32 * (p0 + np_)],
                              ps1[:, 32 * p0:32 * (p0 + np_)])

    w2tt_s = pool.tile([128, 1024], bf16, tag="w2tt")
    out_s = pool.tile([128, 1024], bf16, tag="out")
    ps2 = ps_pool.tile([128, 1024], f32, tag="s2")
    w2t_v = w2t_s[:].rearrange("p (tg c2) -> p tg c2", c2=8)
    if v19:
        # seed every ps2 block with bias via rank-1 matmul (ones x bias row)
        # in the idle PE window; step-2 matmuls then accumulate on top
        for c2 in range(8):
            nc.tensor.matmul(
                ps2[:, 128 * c2:128 * (c2 + 1)],
                ones_s[:, 0:128],
                in_s[0:1, 6144:6272],
                start=True, stop=False,
            )
    for c2 in range(8):
        pst = tp_pool.tile([128, 128], bf16, tag="tp")
        nc.tensor.transpose(pst[:], w2t_v[:, :, c2:c2 + 1], ident[:])
        if v16 and c2 % 2 == 0:
            nc.scalar.copy(w2tt_s[:, 128 * c2:128 * (c2 + 1)], pst[:])
        else:
            nc.vector.tensor_copy(w2tt_s[:, 128 * c2:128 * (c2 + 1)], pst[:])
        nc.tensor.matmul(
            ps2[:, 128 * c2:128 * (c2 + 1)],
            x0p[:, 128 * c2:128 * (c2 + 1)],
            w2tt_s[:, 128 * c2:128 * (c2 + 1)],
            start=not v19, stop=True,
        )
        if c2 % 2 == 1:
            u = c2 // 2
            if v19:
                nc.vector.tensor_copy(out_s[:, 256 * u:256 * (u + 1)],
                                      ps2[:, 256 * u:256 * (u + 1)])
            else:
                bias4 = bias_s[:].unsqueeze(1).broadcast_to([128, 2, 128])
                nc.vector.tensor_add(
                    out_s[:, 256 * u:256 * (u + 1)].rearrange(
                        "p (f n) -> p f n", f=2),
                    ps2[:, 256 * u:256 * (u + 1)].rearrange(
                        "p (f n) -> p f n", f=2),
                    bias4,
                )
            (nc.sync if u % 2 == 0 else nc.scalar).dma_start(
                out_d.ap()[:, 256 * u:256 * (u + 1)],
                out_s[:, 256 * u:256 * (u + 1)])


def _build_program(version=None):
    if version is None:
        version = VERSION
    if version in _prog_cache:
        return _prog_cache[version]

    from contextlib import ExitStack

    import concourse.bacc as bacc
    import concourse.mybir as mybir
    import concourse.tile as tile

    f32 = mybir.dt.float32
    nc = bacc.Bacc("TRN2", target_bir_lowering=False, debug=False)

    if version >= 11:
        bf16 = mybir.dt.bfloat16
        in_d = nc.dram_tensor("in_pack", [128, 6272], bf16, kind="ExternalInput")
        out_d = nc.dram_tensor("out_pack", [128, 1024], bf16,
                               kind="ExternalOutput")
        with tile.TileContext(nc) as tc, ExitStack() as ctx:
            pool = ctx.enter_context(tc.tile_pool(name="io", bufs=1))
            ps_pool = ctx.enter_context(
                tc.tile_pool(name="ps", bufs=1, space="PSUM"))
            tp_pool = ctx.enter_context(
                tc.tile_pool(name="tp", bufs=2 if version == 15 else 3,
                             space="PSUM"))
            if version >= 15:
                _emit_body_v15(nc, tc, pool, ps_pool, tp_pool, in_d, out_d,
                               v16=(version == 16),
                               v17=(version >= 17),
                               v18=(version == 18),
                               v19=(version >= 19))
            elif version >= 14:
                _emit_body_v14(nc, tc, pool, ps_pool, tp_pool, in_d, out_d)
            elif version >= 13:
                _emit_body_v13(nc, tc, pool, ps_pool, in_d, out_d)
            elif version >= 12:
                _emit_body_v12(nc, tc, pool, ps_pool, tp_pool, in_d, out_d)
            else:
                _emit_body_v11(nc, tc, pool, ps_pool, tp_pool, in_d, out_d)
        nc.compile()
        _prog_cache[version] = nc
        return nc

    dense = version in (6, 7, 8, 10)
    nx = 512 if dense else 1024
    # in0 = [xk_pack | wt chunk0 (1024)], in1 = [x0_pack | bias (128)]
    in0_d = nc.dram_tensor("in0_pack", [128, nx + 1024], f32, kind="ExternalInput")
    in1_d = nc.dram_tensor("in1_pack", [128, nx + 128], f32, kind="ExternalInput")
    wtr_d = nc.dram_tensor("wtr_pack", [128, 3072], f32, kind="ExternalInput")
    out_d = nc.dram_tensor("out_pack", [128, 1024], f32, kind="ExternalOutput")
    # bounce layout [h, k, j, q, c2, n]
    w2b_d = nc.dram_tensor("w2_bounce", [2, 8, 4, 2, 8, 128], f32)

    with tile.TileContext(nc) as tc, ExitStack() as ctx:
        pool = ctx.enter_context(tc.tile_pool(name="io", bufs=1))
        ps_pool = ctx.enter_context(tc.tile_pool(name="ps", bufs=2, space="PSUM"))
        _emit_body(nc, tc, pool, ps_pool, f32, in0_d, in1_d, wtr_d, out_d, w2b_d,
                   version=version)

    nc.compile()
    _prog_cache[version] = nc
    return nc


def pack_core_inputs(x_0, x_k, conv_w, conv_b, version=None):
    """Returns (in_maps list of 8 dicts) for run_bass_kernel_spmd."""
    if version is None:
        version = VERSION
    if version >= 11:
        import ml_dtypes
        BF = ml_dtypes.bfloat16
        wt = _pack_wt(np.asarray(conv_w, dtype=F32))
        bias = np.broadcast_to(np.asarray(conv_b, dtype=F32), (128, 128))
        x0 = np.asarray(x_0, dtype=F32)
        xk = np.asarray(x_k, dtype=F32)
        in_maps = []
        for r in range(NCORES):
            xk2 = _pack_xk(xk[B * r:B * (r + 1)])
            if version >= 12:
                # block col order (tau, g, c2): newpos 8g+c2 <- c = 2c2+g
                cperm = np.array([2 * (i % 8) + i // 8 for i in range(16)])
                xk2 = xk2.reshape(128, 32, 2, 16)[:, :, :, cperm].reshape(
                    128, 1024)
                x0l = _pack_x0p(x0[B * r:B * (r + 1)])
            else:
                x0l = _pack_x0(x0[B * r:B * (r + 1)])
            b_blk = (np.broadcast_to(
                np.asarray(conv_b, dtype=F32)[:, None], (128, 128))
                if version == 13 else bias)
            in_pack = np.concatenate([xk2, wt, x0l, b_blk], axis=1)
            in_maps.append({"in_pack": np.ascontiguousarray(
                in_pack.astype(BF))})
        return in_maps
    dense = version in (6, 7, 8, 10)
    wt = _pack_wt(np.asarray(conv_w, dtype=F32))
    bias = np.ascontiguousarray(
        np.broadcast_to(np.asarray(conv_b, dtype=F32), (128, 128))
    )
    x0 = np.asarray(x_0, dtype=F32)
    xk = np.asarray(x_k, dtype=F32)
    wtr = np.ascontiguousarray(wt[:, 1024:])  # pairs 8..31, shared by all cores
    in_maps = []
    for r in range(NCORES):
        in0 = np.concatenate(
            [_pack_xk(xk[B * r:B * (r + 1)], dense), wt[:, :1024]], axis=1)
        in1 = np.concatenate(
            [_pack_x0(x0[B * r:B * (r + 1)], dense), bias], axis=1)
        in_maps.append({
            "in0_pack": np.ascontiguousarray(in0),
            "in1_pack": np.ascontiguousarray(in1),
            "wtr_pack": wtr,
        })
    return in_maps


VERSION = 19  # current best variant


def kernel(x_0, x_k, conv_w, conv_b):
    from concourse.bass_utils import run_bass_kernel_spmd

    nc = _build_program(VERSION)
    in_maps = pack_core_inputs(x_0, x_k, conv_w, conv_b, version=VERSION)
    res = run_bass_kernel_spmd(nc, in_maps, core_ids=list(range(NCORES)))
    out = np.empty((BS, NF, F), dtype=F32)
    for r in range(NCORES):
        _unpack_out(np.asarray(res.results[r]["out_pack"], dtype=F32), out, r)
    return out


# ---------------------------------------------------------------------------
# numpy model of the packed device program (for testing the packing logic)
# ---------------------------------------------------------------------------

def _numpy_model(x_0, x_k, conv_w, conv_b):
    out = np.empty((BS, NF, F), dtype=F32)
    in_maps = pack_core_inputs(x_0, x_k, conv_w, conv_b, version=2)
    for r in range(NCORES):
        m = in_maps[r]
        xk_s = m["in0_pack"][:, :1024]
        wt = np.concatenate([m["in0_pack"][:, 1024:], m["wtr_pack"]], axis=1)
        x0l = m["in1_pack"][:, :1024]
        bias = m["in1_pack"][:, 1024:1152]
        w2 = np.zeros((128, 1024), dtype=F32)
        for k in range(8):
            ps1 = np.zeros((128, 128), dtype=F32)
            for j in range(4):
                p = 4 * k + j
                ps1[32 * j:32 * (j + 1), :] = (
                    xk_s[:, 32 * p:32 * (p + 1)].T @ wt[:, 128 * p:128 * (p + 1)]
                )
            w2[:, 128 * k:128 * (k + 1)] = ps1
        # bounce: src partition (j,q,c2,h), free (k,n) -> dst [h,k,j,q,c2,n]
        srcA = w2.reshape(4, 2, 8, 2, 8, 128)          # [j,q,c2,h,k,n]
        w2b = srcA.transpose(3, 4, 0, 1, 2, 5)         # [h,k,j,q,c2,n]
        w2r = w2b.reshape(128, 8, 128).reshape(128, 1024)  # partition (h,k,j,q)
        out_pack = np.empty((128, 1024), dtype=F32)
        for c2 in range(8):
            out_pack[:, 128 * c2:128 * (c2 + 1)] = (
                x0l[:, 128 * c2:128 * (c2 + 1)].T @ w2r[:, 128 * c2:128 * (c2 + 1)]
                + bias
            )
        _unpack_out(out_pack, out, r)
    return out



# revision 25
# speedup vs baseline: 1.1492x; 1.1213x over previous
"""Trainium2 Bass kernel for the CIN-style layer:

    z   = einsum('btf,byf->bfty', x_0, x_k)            # pairwise outer products
    z   = z.reshape(bs, ts0, f, tsk)                   # flat reinterpretation
    out = einsum('btiy,nty->bni', z, conv_w) + conv_b  # strided conv reduction

Shapes: x_0 (32, 64, 256), x_k (32, 64, 256), conv_w (128, 64, 64),
conv_b (128,) -> out (32, 128, 256).

Math: with i = a*64 + m (a = i//64, m = i%64) and feature f = 4t + a the
reference reduces to a two-step factorization (~270 MFLOP vs 8.6 GFLOP naive):

    W2[b,n,t,a]      = sum_y x_k[b,y,4t+a] * conv_w[n,t,y]         (contract y)
    out[b,n,a*64+m]  = sum_t x_0[b,m,4t+a] * W2[b,n,t,a] + conv_b  (contract t)

Sharding: pure data parallel over batch, 4 samples per core, conv_w/conv_b
replicated (no collectives).

Shipped variant (VERSION=17 = v15 + output eighths on both rings +
3-deep transpose psum pool; all device data bf16, rel err ~5e-3 vs the
2e-2 gate; fp32 baseline v3 was 36us on this box, v15/v17 are ~23.7-24us):
  step 1 (flipped vs v3): stationary = dense conv_w pair-tile
      [K=128 (tau,y), M=128 n], moving = block-diag xk tile [128, 32] ->
      PSUM W2T [n | 8*(2t+g)+c2] (c = 2*c2+g = 4b+a), 32 matmuls writing
      contiguous 32-col slices; per-group contiguous fp32->bf16 casts.
      This kills v3's 9us W2 DRAM bounce outright.
  transpose: 8 PE transposes, each reading one stride-8 single-free-dim
      slice [n | (2t+g)] of W2T (BIR requires one free dim on the
      stationary) -> PSUM [2t+g | n] bf16, copied to SBUF on vector.
  step 2: stationary = zero-padded interleaved x0 tile [K=128 (2t+g),
      M=128 (g',m)], moving = transposed W2 [128, 128 n] -> PSUM
      [64g'+m | (c2, n)]; bias fused into the PSUM->SBUF add; output
      shipped bf16 in eighths issued pairwise on both HWDGE rings and
      upcast to fp32 on the host.
  DMA: input is one [128, 6272] bf16 tensor (xk2 | wt2 | x0p | bias),
      six 256KB-ish chunks spread over the sync/scalar HWDGE rings and
      the gpsimd software-DGE ring; step-1 groups are emitted in chunk
      arrival order (1,3,0,2). 8 bf16 warmup matmuls cover the DMA window.

Known dead ends kept for reference: v13 (matmul is_transpose with a
non-identity moving operand is NOT a fused transposed-matmul - the moving
side must be a permutation matrix, results were garbage), v14 (split-K
step 2 with partition-offset transposes broke correctness), v16 (scalar
ACTIVATE copies + table preload measured slower - the preload was emitted
before scalar's DMA triggers and delayed the whole ring), v18 (splitting
the last weight chunk + its cast into halves measured slower), v19
(seeding bias into PSUM via a rank-1 matmul with accumulating step-2
matmuls measured slower and doubled rel err).

All layout work happens host-side in numpy; the device only runs
contiguous DMAs, matmuls, PE transposes, casts and copies.
"""

import numpy as np

BS, TS, F, NF = 32, 64, 256, 128
NCORES = 8
B = BS // NCORES  # 4 local batches per core

F32 = np.float32


# ---------------------------------------------------------------------------
# Host-side packing
# ---------------------------------------------------------------------------

def _pack_wt(conv_w: np.ndarray) -> np.ndarray:
    # WT[64q+y, 128p+n] = conv_w[n, 2p+q, y]
    wt = conv_w.transpose(1, 2, 0).reshape(32, 2, 64, NF)  # [p, q, y, n]
    wt = wt.transpose(1, 2, 0, 3)                          # [q, y, p, n]
    return np.ascontiguousarray(wt.reshape(128, 32 * NF), dtype=F32)


def _pack_xk(xk_shard: np.ndarray, dense=False) -> np.ndarray:
    # padded: XK[64q+y, 32p+16q'+c] = xk[b, y, 8p+4q+a] iff q'==q else 0
    # dense:  XKD[64q+y, 16p+c]     = xk[b, y, 8p+4q+a]          (c = 4b+a)
    xq = xk_shard.reshape(B, TS, 32, 2, 4)       # [b, y, p, q, a]
    src = xq.transpose(3, 1, 2, 0, 4)            # [q, y, p, b, a]
    if dense:
        return np.ascontiguousarray(src.reshape(128, 512))
    arr = np.zeros((2, TS, 32, 2, B, 4), dtype=F32)
    arr[0, :, :, 0] = src[0]
    arr[1, :, :, 1] = src[1]
    return arr.reshape(128, 32 * 32)


def _pack_x0(x0_shard: np.ndarray, dense=False) -> np.ndarray:
    # padded: X0L[64h+t, 128c2+64h'+m] = x0[b(c), m, 4t+a(c)] iff h'==h
    # dense:  X0D[64h+t, 64c2+m]       = x0[b(c), m, 4t+a(c)]   (c = 2*c2+h)
    xt = x0_shard.reshape(B, TS, TS, 4).transpose(0, 3, 2, 1)  # [b, a, t, m]
    flat = xt.reshape(16, TS, TS)                              # [c, t, m]
    if dense:
        arr = np.zeros((2, TS, 8, TS), dtype=F32)              # [h, t, c2, m]
        for h in (0, 1):
            arr[h] = flat[2 * np.arange(8) + h].transpose(1, 0, 2)
        return arr.reshape(128, 512)
    arr = np.zeros((2, TS, 8, 2, TS), dtype=F32)               # [h, t, c2, h', m]
    for h in (0, 1):
        arr[h, :, :, h, :] = flat[2 * np.arange(8) + h].transpose(1, 0, 2)
    return arr.reshape(128, 8 * 128)


def _unpack_out(out_pack: np.ndarray, out_full: np.ndarray, r: int) -> None:
    if VERSION == 13:
        # out_pack[n, 128c2+64g+m] = out[4r+b(c), n, a(c)*64+m], c = 2*c2+g
        o = out_pack.reshape(NF, 8, 2, TS)  # [n, c2, g, m]
        for c2 in range(8):
            for g in (0, 1):
                c = 2 * c2 + g
                b, a = divmod(c, 4)
                out_full[4 * r + b, :, a * TS:(a + 1) * TS] = o[:, c2, g, :]
        return
    # out_pack[64h+m, 128c2+n] = out[4r+b(c), n, a(c)*64+m], c = 2*c2+h
    o = out_pack.reshape(2, TS, 8, NF)  # [h, m, c2, n]
    for c2 in range(8):
        for h in (0, 1):
            c = 2 * c2 + h
            b, a = divmod(c, 4)
            out_full[4 * r + b, :, a * TS:(a + 1) * TS] = o[h, :, c2, :].T


# ---------------------------------------------------------------------------
# Device program
# ---------------------------------------------------------------------------

_prog_cache = {}


def _emit_body(nc, tc, pool, ps_pool, f32, in0_d, in1_d, wtr_d, out_d, w2b_d,
               version=2, stage="all"):
    # stage: "in" = input DMAs only, "s1" = through step-1 copies,
    #        "shuffle" = through the bounce, "all" = full kernel
    import concourse.mybir as mybir

    if version >= 3:
        # PE warm-up: ~3.4us of back-to-back matmuls on a zeroed bf16 tile
        # while the input DMAs stream in; gets the HAM clock gate to 2.4GHz
        # before step 1 starts.  No data deps -> scheduled first on PE.
        warm_s = pool.tile([128, 512], mybir.dt.bfloat16, tag="warm")
        nc.gpsimd.memset(warm_s[:], 0.0)
        ps_w = ps_pool.tile([128, 512], f32, tag="warm_ps")
        for _ in range(8):
            nc.tensor.matmul(ps_w[:, :], warm_s[:, 0:128], warm_s[:, :],
                             start=True, stop=True)

    dense = version in (6, 7, 8, 10)   # xk/x0 shipped dense, padded on-chip
    merged = version in (5, 7, 8, 9)   # single wtr DMA + single out DMA
    dual = version in (8, 9, 10)       # use both HWDGE rings (SP + ACT)
    eng2 = nc.scalar if dual else nc.sync

    nxk = 512 if dense else 1024   # xk cols in in0
    nx0 = 512 if dense else 1024   # x0 cols in in1
    in0_s = pool.tile([128, nxk + 1024], f32, tag="in0")
    nc.sync.dma_start(in0_s[:], in0_d.ap())
    wtr_s = []
    if merged:
        t_ = pool.tile([128, 3072], f32, tag="wtr")
        eng2.dma_start(t_[:], wtr_d.ap())
        wtr_s = [t_[:, 0:1024], t_[:, 1024:2048], t_[:, 2048:3072]]
    else:
        chunk_eng = [eng2, nc.sync, eng2]
        for chunk in range(3):
            t_ = pool.tile([128, 1024], f32, tag=f"wtr{chunk}")
            chunk_eng[chunk].dma_start(
                t_[:], wtr_d.ap()[:, 1024 * chunk:1024 * (chunk + 1)])
            wtr_s.append(t_[:])
    in1_s = pool.tile([128, nx0 + 128], f32, tag="in1")
    nc.sync.dma_start(in1_s[:], in1_d.ap())

    if dense:
        # zero-pad dense xk/x0 into block-diagonal lhsT layouts on-chip
        # (memsets + strided DVE copies are hidden under the DMA stream)
        xk_pad = pool.tile([128, 1024], f32, tag="xkpad")
        nc.gpsimd.memset(xk_pad[:], 0.0)
        x0_pad = pool.tile([128, 1024], f32, tag="x0pad")
        nc.gpsimd.memset(x0_pad[:], 0.0)
        for q in range(2):
            dst = xk_pad[64 * q:64 * (q + 1), :].rearrange(
                "p (a b) -> p a b", b=32)[:, :, 16 * q:16 * (q + 1)]
            src = in0_s[64 * q:64 * (q + 1), 0:512].rearrange(
                "p (a b) -> p a b", b=16)
            nc.vector.tensor_copy(dst, src)
        for h in range(2):
            dst = x0_pad[64 * h:64 * (h + 1), :].rearrange(
                "p (a b) -> p a b", b=128)[:, :, 64 * h:64 * (h + 1)]
            src = in1_s[64 * h:64 * (h + 1), 0:512].rearrange(
                "p (a b) -> p a b", b=64)
            nc.vector.tensor_copy(dst, src)
        xk_s = xk_pad[:, 0:1024]
        x0_s = x0_pad[:, 0:1024]
    else:
        xk_s = in0_s[:, 0:1024]
        x0_s = in1_s[:, 0:1024]
    bias_s = in1_s[:, nx0:nx0 + 128]

    def wt_cols(p):  # rhs tile [128, 128] for pair p
        if p < 8:
            return in0_s[:, nxk + 128 * p:nxk + 128 * (p + 1)]
        chunk, off = divmod(128 * (p - 8), 1024)
        return wtr_s[chunk][:, off:off + 128]

    if stage == "in":
        return

    # ---- step 1: W2 = xk . wT, contract y (K = 128 = (q, y)) ----
    # psum tile u holds passes 4u..4u+3 at col 128*(k%4)
    # ---- shuffle (q,c)-partitioned W2 -> t-partitioned via DRAM bounce ----
    # (a direct SBUF->SBUF partition-gather is impossible: the BIR verifier
    # rejects partition steps != 1 and the permutation needs >3 AP dims)
    w2_s = pool.tile([128, 1024], f32, tag="w2")
    w2r_s = pool.tile([128, 1024], f32, tag="w2r")

    def emit_pass(k, ps1):
        for j in range(4):
            p = 4 * k + j
            nc.tensor.matmul(
                ps1[32 * j:32 * (j + 1), 128 * (k % 4):128 * (k % 4 + 1)],
                xk_s[:, 32 * p:32 * (p + 1)],
                wt_cols(p),
                start=True,
                stop=True,
                tile_position=(0, 32 * j),
            )

    if version == 4:
        # per-pass copy + per-pass bounce-out (k fixed -> <=3 AP dims), so
        # all but the last bounce hides under step 1; readback in (h, k-half)
        # quarters, the first two of which also overlap step 1.
        # (measured WORSE on HW: per-DMA serialized overhead dominates)
        for u in range(2):
            ps1 = ps_pool.tile([128, 512], f32, tag="s1")
            for k in range(4 * u, 4 * u + 4):
                emit_pass(k, ps1)
                kk = 128 * (k % 4)
                nc.vector.tensor_copy(w2_s[:, 128 * k:128 * (k + 1)],
                                      ps1[:, kk:kk + 128])
                dstA = w2b_d.ap()[:, k].rearrange("h j q c2 n -> j q c2 h n")
                nc.sync.dma_start(dstA, w2_s[:, 128 * k:128 * (k + 1)])
            for h in range(2):
                kh = u
                dstB = w2r_s[64 * h + 32 * kh:64 * h + 32 * kh + 32, :]
                nc.sync.dma_start(dstB, w2b_d.ap()[h, 4 * kh:4 * (kh + 1)])
    else:
        for u in range(2):
            ps1 = ps_pool.tile([128, 512], f32, tag="s1")
            for k in range(4 * u, 4 * u + 4):
                emit_pass(k, ps1)
            nc.vector.tensor_copy(w2_s[:, 512 * u:512 * (u + 1)], ps1[:, :])
        if stage == "s1":
            return
        srcA = w2_s[:].rearrange("p (k n) -> p k n", k=8)
        dstA = w2b_d.ap().rearrange("h k j q c2 n -> j q c2 h k n")
        eng2.dma_start(dstA, srcA)
        dstB = w2r_s[:].rearrange("p (c2 n) -> p c2 n", c2=8)
        nc.sync.dma_start(dstB, w2b_d.ap())
        if stage == "shuffle":
            return

    # ---- step 2: out = x0 . W2, contract t (K = 128 = (h, t)) ----
    out_s = pool.tile([128, 1024], f32, tag="out")
    for u in range(2):
        ps2 = ps_pool.tile([128, 512], f32, tag="s2")
        for c2 in range(4 * u, 4 * u + 4):
            nc.tensor.matmul(
                ps2[:, 128 * (c2 % 4):128 * (c2 % 4 + 1)],
                x0_s[:, 128 * c2:128 * (c2 + 1)],
                w2r_s[:, 128 * c2:128 * (c2 + 1)],
                start=True,
                stop=True,
            )
        bias4 = bias_s.unsqueeze(1).broadcast_to([128, 4, 128])
        nc.vector.tensor_add(
            out_s[:, 512 * u:512 * (u + 1)].rearrange("p (f n) -> p f n", f=4),
            ps2[:, :].rearrange("p (f n) -> p f n", f=4),
            bias4,
        )
        if version >= 3 and not merged:
            (nc.sync if u == 0 else eng2).dma_start(
                out_d.ap()[:, 512 * u:512 * (u + 1)],
                out_s[:, 512 * u:512 * (u + 1)])
    if version == 2 or merged:
        eng2.dma_start(out_d.ap(), out_s[:])


def _pack_x0p(x0_shard: np.ndarray) -> np.ndarray:
    # x0p[2t+g, 128c2+64g'+m] = x0[b(c), m, 4t+a(c)] iff g==g', c = 2c2+g = 4b+a
    arr = np.zeros((TS, 2, 8, 2, TS), dtype=F32)  # [t, g, c2, g', m]
    for c in range(16):
        c2, g = divmod(c, 2)
        b, a = divmod(c, 4)
        arr[:, g, c2, g, :] = x0_shard[b, :, a::4].T  # [t, m]
    return arr.reshape(128, 1024)


def _emit_body_v11(nc, tc, pool, ps_pool, tp_pool, in_d, out_d, warm=8):
    """bf16 bounce-free pipeline.

    step 1 (flipped vs v3): stationary = dense conv_w pair-tile
      [K=128 (tau,y), M=128 n], moving = block-diag xk tile [128, 32 (tau',c)]
      -> PSUM W2T [n | 32p+16tau'+c] = [n | (t, c)], 32 matmuls, no bounce.
    transpose: 8 PE transposes of strided slices [n | (t, g)] (g = c parity,
      c = 2*c2+g) -> PSUM [2t+g | n] per c2, bf16.
    step 2: stationary = block-diag x0 tile [K=128 (2t+g), M=128 (g',m)],
      moving = transposed W2 [128, 128 n] -> PSUM [64g'+m | (c2, n)], the v3
      out_pack layout. Bias added in the PSUM->SBUF copy; output DMA'd bf16.
    """
    import concourse.mybir as mybir
    from concourse import masks

    bf16 = mybir.dt.bfloat16
    f32 = mybir.dt.float32

    # PE warm-up on a zeroed bf16 tile while input DMAs stream (p-state ramp)
    warm_s = pool.tile([128, 512], bf16, tag="warm")
    nc.gpsimd.memset(warm_s[:], 0.0)
    ps_w = ps_pool.tile([128, 512], f32, tag="warm_ps")
    for _ in range(warm):
        nc.tensor.matmul(ps_w[:, :], warm_s[:, 0:128], warm_s[:, :],
                         start=True, stop=True)

    ident = pool.tile([128, 128], bf16, tag="ident")
    masks.make_identity(nc, ident[:])

    # in_s cols: [0:1024 xk2 | 1024:5120 wt2 | 5120:6144 x0p | 6144:6272 bias]
    in_s = pool.tile([128, 6272], bf16, tag="in")
    nc.sync.dma_start(in_s[:, 0:1024], in_d.ap()[:, 0:1024])        # xk2
    nc.sync.dma_start(in_s[:, 1024:2048], in_d.ap()[:, 1024:2048])  # wt2 p0-7
    nc.sync.dma_start(in_s[:, 2048:3072], in_d.ap()[:, 2048:3072])  # wt2 p8-15
    nc.scalar.dma_start(in_s[:, 3072:4096], in_d.ap()[:, 3072:4096])  # p16-23
    nc.scalar.dma_start(in_s[:, 4096:5120], in_d.ap()[:, 4096:5120])  # p24-31
    nc.scalar.dma_start(in_s[:, 5120:6272], in_d.ap()[:, 5120:6272])  # x0p+bias

    xk2 = in_s[:, 0:1024]
    x0p = in_s[:, 5120:6144]
    bias_bf = in_s[:, 6144:6272]
    if v19:
        ones_s = pool.tile([1, 128], bf16, tag="ones")
        nc.gpsimd.memset(ones_s[:], 1.0)
    else:
        bias_s = pool.tile([128, 128], f32, tag="bias32")
        nc.gpsimd.tensor_copy(bias_s[:], bias_bf)  # upcast once for the DVE add

    # ---- step 1: W2T[n, 32p+16tau+c] -> psum [128, 1024] fp32 ----
    w2t_s = pool.tile([128, 1024], bf16, tag="w2t")
    ps1 = ps_pool.tile([128, 1024], f32, tag="s1")
    # pair groups ordered to match DMA-chunk arrival (sync: 0-7, 8-15 after
    # xk2; scalar ring delivers 16-23 earliest)
    for grp in (2, 0, 3, 1):
        for p in range(8 * grp, 8 * grp + 8):
            nc.tensor.matmul(
                ps1[:, 32 * p:32 * (p + 1)],
                in_s[:, 1024 + 128 * p:1024 + 128 * (p + 1)],
                xk2[:, 32 * p:32 * (p + 1)],
                start=True, stop=True,
            )
        dst = w2t_s[:].rearrange("p (c t) -> p t c", t=64)[
            :, 16 * grp:16 * (grp + 1), :]
        src = ps1[:, 256 * grp:256 * (grp + 1)].rearrange(
            "p (t c) -> p t c", c=16)
        nc.vector.tensor_copy(dst, src)

    # ---- transpose + step 2, pipelined per c2 ----
    w2tt_s = pool.tile([128, 1024], bf16, tag="w2tt")
    out_s = pool.tile([128, 1024], bf16, tag="out")
    ps2 = ps_pool.tile([128, 1024], f32, tag="s2")
    for c2 in range(8):
        pst = tp_pool.tile([128, 128], bf16, tag="tp")
        nc.tensor.transpose(pst[:], w2t_s[:, 128 * c2:128 * (c2 + 1)],
                            ident[:])
        if c2 % 2:
            nc.scalar.copy(w2tt_s[:, 128 * c2:128 * (c2 + 1)], pst[:])
        else:
            nc.vector.tensor_copy(w2tt_s[:, 128 * c2:128 * (c2 + 1)], pst[:])
        nc.tensor.matmul(
            ps2[:, 128 * c2:128 * (c2 + 1)],
            x0p[:, 128 * c2:128 * (c2 + 1)],
            w2tt_s[:, 128 * c2:128 * (c2 + 1)],
            start=True, stop=True,
        )
        if c2 == 3 or c2 == 7:
            u = c2 // 4
            bias4 = bias_s[:].unsqueeze(1).broadcast_to([128, 4, 128])
            nc.vector.tensor_add(
                out_s[:, 512 * u:512 * (u + 1)].rearrange(
                    "p (f n) -> p f n", f=4),
                ps2[:, 512 * u:512 * (u + 1)].rearrange(
                    "p (f n) -> p f n", f=4),
                bias4,
            )
            (nc.sync if u == 0 else nc.scalar).dma_start(
                out_d.ap()[:, 512 * u:512 * (u + 1)],
                out_s[:, 512 * u:512 * (u + 1)])


def _emit_body_v12(nc, tc, pool, ps_pool, tp_pool, in_d, out_d, warm=8):
    """v11 + reordered W2T columns (8*(2t+g)+c2) so each c2 transpose input
    is one stride-8 free dim; single contiguous fp32->bf16 cast; vector-only
    PSUM copies (no ACT table load); gpsimd as third input DMA ring."""
    import concourse.mybir as mybir
    from concourse import masks

    bf16 = mybir.dt.bfloat16
    f32 = mybir.dt.float32

    warm_s = pool.tile([128, 512], bf16, tag="warm")
    nc.gpsimd.memset(warm_s[:], 0.0)
    ps_w = ps_pool.tile([128, 512], f32, tag="warm_ps")
    for _ in range(warm):
        nc.tensor.matmul(ps_w[:, :], warm_s[:, 0:128], warm_s[:, :],
                         start=True, stop=True)

    ident = pool.tile([128, 128], bf16, tag="ident")
    masks.make_identity(nc, ident[:])

    # in_s cols: [0:1024 xk2 | 1024:5120 wt2 | 5120:6144 x0p | 6144:6272 bias]
    in_s = pool.tile([128, 6272], bf16, tag="in")
    nc.sync.dma_start(in_s[:, 0:1024], in_d.ap()[:, 0:1024])          # xk2
    nc.sync.dma_start(in_s[:, 1024:2048], in_d.ap()[:, 1024:2048])    # p0-7
    nc.sync.dma_start(in_s[:, 2048:3072], in_d.ap()[:, 2048:3072])    # p8-15
    nc.scalar.dma_start(in_s[:, 3072:4096], in_d.ap()[:, 3072:4096])  # p16-23
    nc.scalar.dma_start(in_s[:, 4096:5120], in_d.ap()[:, 4096:5120])  # p24-31
    nc.gpsimd.dma_start(in_s[:, 5120:6272], in_d.ap()[:, 5120:6272])  # x0p+b

    xk2 = in_s[:, 0:1024]
    x0p = in_s[:, 5120:6144]
    bias_bf = in_s[:, 6144:6272]
    if v19:
        ones_s = pool.tile([1, 128], bf16, tag="ones")
        nc.gpsimd.memset(ones_s[:], 1.0)
    else:
        bias_s = pool.tile([128, 128], f32, tag="bias32")
        nc.gpsimd.tensor_copy(bias_s[:], bias_bf)

    # ---- step 1: psum cols 32p+16tau+(8g+c2), contiguous writes ----
    w2t_s = pool.tile([128, 1024], bf16, tag="w2t")
    ps1 = ps_pool.tile([128, 1024], f32, tag="s1")
    for grp in (2, 0, 3, 1):
        for p in range(8 * grp, 8 * grp + 8):
            nc.tensor.matmul(
                ps1[:, 32 * p:32 * (p + 1)],
                in_s[:, 1024 + 128 * p:1024 + 128 * (p + 1)],
                xk2[:, 32 * p:32 * (p + 1)],
                start=True, stop=True,
            )
    nc.vector.tensor_copy(w2t_s[:], ps1[:])  # one contiguous cast

    # ---- transpose (stride-8 read) + step 2, pipelined per c2 ----
    w2tt_s = pool.tile([128, 1024], bf16, tag="w2tt")
    out_s = pool.tile([128, 1024], bf16, tag="out")
    ps2 = ps_pool.tile([128, 1024], bf16, tag="s2")
    w2t_v = w2t_s[:].rearrange("p (tg c2) -> p tg c2", c2=8)
    for c2 in range(8):
        pst = tp_pool.tile([128, 128], bf16, tag="tp")
        nc.tensor.transpose(pst[:], w2t_v[:, :, c2:c2 + 1], ident[:])
        if v16 and c2 % 2 == 0:
            nc.scalar.copy(w2tt_s[:, 128 * c2:128 * (c2 + 1)], pst[:])
        else:
            nc.vector.tensor_copy(w2tt_s[:, 128 * c2:128 * (c2 + 1)], pst[:])
        nc.tensor.matmul(
            ps2[:, 128 * c2:128 * (c2 + 1)],
            x0p[:, 128 * c2:128 * (c2 + 1)],
            w2tt_s[:, 128 * c2:128 * (c2 + 1)],
            start=True, stop=True,
        )
        if c2 == 3 or c2 == 7:
            u = c2 // 4
            bias4 = bias_s[:].unsqueeze(1).broadcast_to([128, 4, 128])
            nc.vector.tensor_add(
                out_s[:, 512 * u:512 * (u + 1)].rearrange(
                    "p (f n) -> p f n", f=4),
                ps2[:, 512 * u:512 * (u + 1)].rearrange(
                    "p (f n) -> p f n", f=4),
                bias4,
            )
            (nc.sync if u == 0 else nc.scalar).dma_start(
                out_d.ap()[:, 512 * u:512 * (u + 1)],
                out_s[:, 512 * u:512 * (u + 1)])


def _emit_body_v13(nc, tc, pool, ps_pool, in_d, out_d, warm=8):
    """v12 + transpose fused into step 2: matmul(is_transpose=True) loads the
    W2T stride-8 slice transposed as stationary and streams x0p as moving ->
    out [n | (c2, g, m)] with no PE transposes, no tp psum tiles, no copies.
    Casts are per-group (contiguous). Bias is per-partition (n)."""
    import concourse.mybir as mybir

    bf16 = mybir.dt.bfloat16
    f32 = mybir.dt.float32

    warm_s = pool.tile([128, 512], bf16, tag="warm")
    nc.gpsimd.memset(warm_s[:], 0.0)
    ps_w = ps_pool.tile([128, 512], f32, tag="warm_ps")
    for _ in range(warm):
        nc.tensor.matmul(ps_w[:, :], warm_s[:, 0:128], warm_s[:, :],
                         start=True, stop=True)

    # in_s cols: [0:1024 xk2 | 1024:5120 wt2 | 5120:6144 x0p | 6144:6272 bias]
    in_s = pool.tile([128, 6272], bf16, tag="in")
    nc.sync.dma_start(in_s[:, 0:1024], in_d.ap()[:, 0:1024])          # xk2
    nc.sync.dma_start(in_s[:, 1024:2048], in_d.ap()[:, 1024:2048])    # p0-7
    nc.scalar.dma_start(in_s[:, 2048:3072], in_d.ap()[:, 2048:3072])  # p8-15
    nc.scalar.dma_start(in_s[:, 3072:4096], in_d.ap()[:, 3072:4096])  # p16-23
    nc.gpsimd.dma_start(in_s[:, 4096:5120], in_d.ap()[:, 4096:5120])  # p24-31
    nc.gpsimd.dma_start(in_s[:, 5120:6272], in_d.ap()[:, 5120:6272])  # x0p+b

    xk2 = in_s[:, 0:1024]
    x0p = in_s[:, 5120:6144]
    bias_bf = in_s[:, 6144:6272]   # [n, j] = conv_b[n] (row-indexed)
    bias_s = pool.tile([128, 128], f32, tag="bias32")
    nc.gpsimd.tensor_copy(bias_s[:], bias_bf)

    # ---- step 1 + per-group contiguous casts ----
    w2t_s = pool.tile([128, 1024], bf16, tag="w2t")
    ps1 = ps_pool.tile([128, 1024], f32, tag="s1")
    # (start_pair, n_pairs) subgroups in chunk-arrival order; each is
    # followed by a contiguous cast of just its psum columns
    subgroups = ([(8, 8), (24, 8), (16, 4), (0, 8), (20, 4)] if v18
                 else [(8, 8), (24, 8), (0, 8), (16, 8)])
    for p0, np_ in subgroups:
        for p in range(p0, p0 + np_):
            nc.tensor.matmul(
                ps1[:, 32 * p:32 * (p + 1)],
                in_s[:, 1024 + 128 * p:1024 + 128 * (p + 1)],
                xk2[:, 32 * p:32 * (p + 1)],
                start=True, stop=True,
            )
        nc.vector.tensor_copy(w2t_s[:, 32 * p0:32 * (p0 + np_)],
                              ps1[:, 32 * p0:32 * (p0 + np_)])

    # ---- step 2: transposed-load W2T slice (stride 8) x moving x0p ----
    out_s = pool.tile([128, 1024], bf16, tag="out")
    ps2 = ps_pool.tile([128, 1024], bf16, tag="s2")
    w2t_v = w2t_s[:].rearrange("p (tg c2) -> p tg c2", c2=8)
    for c2 in range(8):
        nc.tensor.matmul(
            ps2[:, 128 * c2:128 * (c2 + 1)],
            w2t_v[:, :, c2:c2 + 1],
            x0p[:, 128 * c2:128 * (c2 + 1)],
            is_transpose=True,
            start=True, stop=True,
        )
        if c2 == 3 or c2 == 7:
            u = c2 // 4
            bias_b = bias_s[:, 0:1].broadcast_to([128, 512])
            nc.vector.tensor_add(
                out_s[:, 512 * u:512 * (u + 1)],
                ps2[:, 512 * u:512 * (u + 1)],
                bias_b,
            )
            (nc.sync if u == 0 else nc.scalar).dma_start(
                out_d.ap()[:, 512 * u:512 * (u + 1)],
                out_s[:, 512 * u:512 * (u + 1)])


def _emit_body_v14(nc, tc, pool, ps_pool, tp_pool, in_d, out_d, warm=8):
    """v13 DMA/cast structure + v12-style PE transposes, split-K step 2:
    each c2 accumulates two t-halves in PSUM, so half-A transposes+matmuls
    overlap the tail of the weight stream. Quarter-granularity output."""
    import concourse.mybir as mybir
    from concourse import masks

    bf16 = mybir.dt.bfloat16
    f32 = mybir.dt.float32

    warm_s = pool.tile([128, 512], bf16, tag="warm")
    nc.gpsimd.memset(warm_s[:], 0.0)
    ps_w = ps_pool.tile([128, 512], f32, tag="warm_ps")
    for _ in range(warm):
        nc.tensor.matmul(ps_w[:, :], warm_s[:, 0:128], warm_s[:, :],
                         start=True, stop=True)

    ident = pool.tile([128, 128], bf16, tag="ident")
    masks.make_identity(nc, ident[:])

    # in_s cols: [0:1024 xk2 | 1024:5120 wt2 | 5120:6144 x0p | 6144:6272 bias]
    in_s = pool.tile([128, 6272], bf16, tag="in")
    ap = in_d.ap()
    nc.sync.dma_start(in_s[:, 0:1024], ap[:, 0:1024])                # xk2
    nc.scalar.dma_start(in_s[:, 1024:2048], ap[:, 1024:2048])        # p0-7
    nc.gpsimd.dma_start(in_s[0:64, 5120:6144], ap[0:64, 5120:6144])  # x0p top
    nc.gpsimd.dma_start(in_s[:, 2048:3072], ap[:, 2048:3072])        # p8-15
    nc.sync.dma_start(in_s[:, 3072:4096], ap[:, 3072:4096])          # p16-23
    nc.scalar.dma_start(in_s[:, 4096:5120], ap[:, 4096:5120])        # p24-31
    nc.sync.dma_start(in_s[64:128, 5120:6144], ap[64:128, 5120:6144])  # x0p bot
    nc.scalar.dma_start(in_s[:, 6144:6272], ap[:, 6144:6272])        # bias

    xk2 = in_s[:, 0:1024]
    x0p = in_s[:, 5120:6144]
    bias_bf = in_s[:, 6144:6272]
    if v19:
        ones_s = pool.tile([1, 128], bf16, tag="ones")
        nc.gpsimd.memset(ones_s[:], 1.0)
    else:
        bias_s = pool.tile([128, 128], f32, tag="bias32")
        nc.gpsimd.tensor_copy(bias_s[:], bias_bf)

    w2t_s = pool.tile([128, 1024], bf16, tag="w2t")
    ps1 = ps_pool.tile([128, 1024], f32, tag="s1")
    w2tt_s = pool.tile([128, 1024], bf16, tag="w2tt")
    out_s = pool.tile([128, 1024], bf16, tag="out")
    if v20:
        # two bank-aligned tiles: bias-adds reading the low half no longer
        # impose a false whole-tile WAR on matmuls writing the high half
        ps2a = ps_pool.tile([128, 512], f32, tag="s2a")
        ps2b = ps_pool.tile([128, 512], f32, tag="s2b")
        ps2_of = lambda c2: (ps2a if c2 < 4 else ps2b, (c2 % 4) * 128)
    else:
        ps2 = ps_pool.tile([128, 1024], f32, tag="s2")
        ps2_of = lambda c2: (ps2, c2 * 128)
    w2t_v = w2t_s[:].rearrange("p (tg c2) -> p tg c2", c2=8)

    def s1_group(grp):
        for p in range(8 * grp, 8 * grp + 8):
            nc.tensor.matmul(
                ps1[:, 32 * p:32 * (p + 1)],
                in_s[:, 1024 + 128 * p:1024 + 128 * (p + 1)],
                xk2[:, 32 * p:32 * (p + 1)],
                start=True, stop=True,
            )
        nc.vector.tensor_copy(w2t_s[:, 256 * grp:256 * (grp + 1)],
                              ps1[:, 256 * grp:256 * (grp + 1)])

    def s2_half(h):
        lo, hi = 64 * h, 64 * (h + 1)
        for c2 in range(8):
            pst = tp_pool.tile([128, 128], bf16, tag="tp")
            nc.tensor.transpose(pst[lo:hi, :], w2t_v[:, lo:hi, c2:c2 + 1],
                                ident[:])
            nc.vector.tensor_copy(w2tt_s[lo:hi, 128 * c2:128 * (c2 + 1)],
                                  pst[lo:hi, :])
            nc.tensor.matmul(
                ps2[:, 128 * c2:128 * (c2 + 1)],
                x0p[lo:hi, 128 * c2:128 * (c2 + 1)],
                w2tt_s[lo:hi, 128 * c2:128 * (c2 + 1)],
                start=(h == 0), stop=(h == 1),
            )
            if h == 1 and c2 % 2 == 1:
                u = c2 // 2
                bias4 = bias_s[:].unsqueeze(1).broadcast_to([128, 2, 128])
                nc.vector.tensor_add(
                    out_s[:, 256 * u:256 * (u + 1)].rearrange(
                        "p (f n) -> p f n", f=2),
                    ps2[:, 256 * u:256 * (u + 1)].rearrange(
                        "p (f n) -> p f n", f=2),
                    bias4,
                )
                (nc.sync if u % 2 == 0 else nc.scalar).dma_start(
                    out_d.ap()[:, 256 * u:256 * (u + 1)],
                    out_s[:, 256 * u:256 * (u + 1)])

    s1_group(0)
    s1_group(1)
    s2_half(0)
    s1_group(2)
    s1_group(3)
    s2_half(1)


def _emit_body_v15(nc, tc, pool, ps_pool, tp_pool, in_d, out_d, warm=8,
                   v16=False, v17=False, v18=False, v19=False, v20=False):
    """v13 DMA/cast structure + v12 transpose step 2 + quarter outputs.
    v16: scalar ACT-table preloaded during warmup, PSUM->SBUF copies
    alternate vector/scalar, deeper transpose pool."""
    import concourse.mybir as mybir
    from concourse import masks

    bf16 = mybir.dt.bfloat16
    f32 = mybir.dt.float32

    warm_s = pool.tile([128, 512], bf16, tag="warm")
    nc.gpsimd.memset(warm_s[:], 0.0)
    ps_w = ps_pool.tile([128, 512], f32, tag="warm_ps")
    for _ in range(warm):
        nc.tensor.matmul(ps_w[:, :], warm_s[:, 0:128], warm_s[:, :],
                         start=True, stop=True)

    ident = pool.tile([128, 128], bf16, tag="ident")
    masks.make_identity(nc, ident[:])
    if v16:
        # touch ACTIVATE during warmup so the 1.3us table load is off the
        # critical path when scalar copies run in the transpose phase
        nc.scalar.copy(warm_s[0:1, 0:1], warm_s[0:1, 1:2])

    in_s = pool.tile([128, 6272], bf16, tag="in")
    ap = in_d.ap()
    nc.sync.dma_start(in_s[:, 0:1024], ap[:, 0:1024])          # xk2
    nc.sync.dma_start(in_s[:, 1024:2048], ap[:, 1024:2048])    # p0-7
    nc.scalar.dma_start(in_s[:, 2048:3072], ap[:, 2048:3072])  # p8-15
    if v18:
        nc.scalar.dma_start(in_s[:, 3072:3584], ap[:, 3072:3584])  # p16-19
        nc.scalar.dma_start(in_s[:, 3584:4096], ap[:, 3584:4096])  # p20-23
    else:
        nc.scalar.dma_start(in_s[:, 3072:4096], ap[:, 3072:4096])  # p16-23
    nc.gpsimd.dma_start(in_s[:, 4096:5120], ap[:, 4096:5120])  # p24-31
    nc.gpsimd.dma_start(in_s[:, 5120:6272], ap[:, 5120:6272])  # x0p+bias

    xk2 = in_s[:, 0:1024]
    x0p = in_s[:, 5120:6144]
    bias_bf = in_s[:, 6144:6272]
    if v19:
        ones_s = pool.tile([1, 128], bf16, tag="ones")
        nc.gpsimd.memset(ones_s[:], 1.0)
    else:
        bias_s = pool.tile([128, 128], f32, tag="bias32")
        nc.gpsimd.tensor_copy(bias_s[:], bias_bf)

    w2t_s = pool.tile([128, 1024], bf16, tag="w2t")
    ps1 = ps_pool.tile([128, 1024], f32, tag="s1")
    # (start_pair, n_pairs) subgroups in chunk-arrival order; each is
    # followed by a contiguous cast of just its psum columns
    subgroups = ([(8, 8), (24, 8), (16, 4), (0, 8), (20, 4)] if v18
                 else [(8, 8), (24, 8), (0, 8), (16, 8)])
    for p0, np_ in subgroups:
        for p in range(p0, p0 + np_):
            nc.tensor.matmul(
                ps1[:, 32 * p:32 * (p + 1)],
                in_s[:, 1024 + 128 * p:1024 + 128 * (p + 1)],
                xk2[:, 32 * p:32 * (p + 1)],
                start=True, stop=True,
            )
        nc.vector.tensor_copy(w2t_s[:, 32 * p0:32 * (p0 + np_)],
                              ps1[:, 32 * p0:32 * (p0 + np_)])

    w2tt_s = pool.tile([128, 1024], bf16, tag="w2tt")
    out_s = pool.tile([128, 1024], bf16, tag="out")
    if v20:
        # two bank-aligned tiles: bias-adds reading the low half no longer
        # impose a false whole-tile WAR on matmuls writing the high half
        ps2a = ps_pool.tile([128, 512], f32, tag="s2a")
        ps2b = ps_pool.tile([128, 512], f32, tag="s2b")
        ps2_of = lambda c2: (ps2a if c2 < 4 else ps2b, (c2 % 4) * 128)
    else:
        ps2 = ps_pool.tile([128, 1024], f32, tag="s2")
        ps2_of = lambda c2: (ps2, c2 * 128)
    w2t_v = w2t_s[:].rearrange("p (tg c2) -> p tg c2", c2=8)
    if v19:
        # seed every ps2 block with bias via rank-1 matmul (ones x bias row)
        # in the idle PE window; step-2 matmuls then accumulate on top
        for c2 in range(8):
            nc.tensor.matmul(
                ps2[:, 128 * c2:128 * (c2 + 1)],
                ones_s[:, 0:128],
                in_s[0:1, 6144:6272],
                start=True, stop=False,
            )
    for c2 in range(8):
        pst = tp_pool.tile([128, 128], bf16, tag="tp")
        nc.tensor.transpose(pst[:], w2t_v[:, :, c2:c2 + 1], ident[:])
        if v16 and c2 % 2 == 0:
            nc.scalar.copy(w2tt_s[:, 128 * c2:128 * (c2 + 1)], pst[:])
        else:
            nc.vector.tensor_copy(w2tt_s[:, 128 * c2:128 * (c2 + 1)], pst[:])
        pt, off = ps2_of(c2)
        nc.tensor.matmul(
            pt[:, off:off + 128],
            x0p[:, 128 * c2:128 * (c2 + 1)],
            w2tt_s[:, 128 * c2:128 * (c2 + 1)],
            start=not v19, stop=True,
        )
        if c2 % 2 == 1:
            u = c2 // 2
            if v19:
                nc.vector.tensor_copy(out_s[:, 256 * u:256 * (u + 1)],
                                      pt[:, off - 128:off + 128])
            else:
                bias4 = bias_s[:].unsqueeze(1).broadcast_to([128, 2, 128])
                nc.vector.tensor_add(
                    out_s[:, 256 * u:256 * (u + 1)].rearrange(
                        "p (f n) -> p f n", f=2),
                    pt[:, off - 128:off + 128].rearrange(
                        "p (f n) -> p f n", f=2),
                    bias4,
                )
            (nc.sync if u % 2 == 0 else nc.scalar).dma_start(
                out_d.ap()[:, 256 * u:256 * (u + 1)],
                out_s[:, 256 * u:256 * (u + 1)])


def _build_program(version=None):
    if version is None:
        version = VERSION
    if version in _prog_cache:
        return _prog_cache[version]

    from contextlib import ExitStack

    import concourse.bacc as bacc
    import concourse.mybir as mybir
    import concourse.tile as tile

    f32 = mybir.dt.float32
    nc = bacc.Bacc("TRN2", target_bir_lowering=False, debug=False)

    if version >= 11:
        bf16 = mybir.dt.bfloat16
        in_d = nc.dram_tensor("in_pack", [128, 6272], bf16, kind="ExternalInput")
        out_d = nc.dram_tensor("out_pack", [128, 1024], bf16,
                               kind="ExternalOutput")
        with tile.TileContext(nc) as tc, ExitStack() as ctx:
            pool = ctx.enter_context(tc.tile_pool(name="io", bufs=1))
            ps_pool = ctx.enter_context(
                tc.tile_pool(name="ps", bufs=1, space="PSUM"))
            tp_pool = ctx.enter_context(
                tc.tile_pool(name="tp", bufs=2 if version == 15 else 3,
                             space="PSUM"))
            if version >= 15:
                _emit_body_v15(nc, tc, pool, ps_pool, tp_pool, in_d, out_d,
                               v16=(version == 16),
                               v17=(version >= 17),
                               v18=(version == 18),
                               v19=(version == 19),
                               v20=(version >= 20))
            elif version >= 14:
                _emit_body_v14(nc, tc, pool, ps_pool, tp_pool, in_d, out_d)
            elif version >= 13:
                _emit_body_v13(nc, tc, pool, ps_pool, in_d, out_d)
            elif version >= 12:
                _emit_body_v12(nc, tc, pool, ps_pool, tp_pool, in_d, out_d)
            else:
                _emit_body_v11(nc, tc, pool, ps_pool, tp_pool, in_d, out_d)
        nc.compile()
        _prog_cache[version] = nc
        return nc

    dense = version in (6, 7, 8, 10)
    nx = 512 if dense else 1024
    # in0 = [xk_pack | wt chunk0 (1024)], in1 = [x0_pack | bias (128)]
    in0_d = nc.dram_tensor("in0_pack", [128, nx + 1024], f32, kind="ExternalInput")
    in1_d = nc.dram_tensor("in1_pack", [128, nx + 128], f32, kind="ExternalInput")
    wtr_d = nc.dram_tensor("wtr_pack", [128, 3072], f32, kind="ExternalInput")
    out_d = nc.dram_tensor("out_pack", [128, 1024], f32, kind="ExternalOutput")
    # bounce layout [h, k, j, q, c2, n]
    w2b_d = nc.dram_tensor("w2_bounce", [2, 8, 4, 2, 8, 128], f32)

    with tile.TileContext(nc) as tc, ExitStack() as ctx:
        pool = ctx.enter_context(tc.tile_pool(name="io", bufs=1))
        ps_pool = ctx.enter_context(tc.tile_pool(name="ps", bufs=2, space="PSUM"))
        _emit_body(nc, tc, pool, ps_pool, f32, in0_d, in1_d, wtr_d, out_d, w2b_d,
                   version=version)

    nc.compile()
    _prog_cache[version] = nc
    return nc


def pack_core_inputs(x_0, x_k, conv_w, conv_b, version=None):
    """Returns (in_maps list of 8 dicts) for run_bass_kernel_spmd."""
    if version is None:
        version = VERSION
    if version >= 11:
        import ml_dtypes
        BF = ml_dtypes.bfloat16
        wt = _pack_wt(np.asarray(conv_w, dtype=F32))
        bias = np.broadcast_to(np.asarray(conv_b, dtype=F32), (128, 128))
        x0 = np.asarray(x_0, dtype=F32)
        xk = np.asarray(x_k, dtype=F32)
        in_maps = []
        for r in range(NCORES):
            xk2 = _pack_xk(xk[B * r:B * (r + 1)])
            if version >= 12:
                # block col order (tau, g, c2): newpos 8g+c2 <- c = 2c2+g
                cperm = np.array([2 * (i % 8) + i // 8 for i in range(16)])
                xk2 = xk2.reshape(128, 32, 2, 16)[:, :, :, cperm].reshape(
                    128, 1024)
                x0l = _pack_x0p(x0[B * r:B * (r + 1)])
            else:
                x0l = _pack_x0(x0[B * r:B * (r + 1)])
            b_blk = (np.broadcast_to(
                np.asarray(conv_b, dtype=F32)[:, None], (128, 128))
                if version == 13 else bias)
            in_pack = np.concatenate([xk2, wt, x0l, b_blk], axis=1)
            in_maps.append({"in_pack": np.ascontiguousarray(
                in_pack.astype(BF))})
        return in_maps
    dense = version in (6, 7, 8, 10)
    wt = _pack_wt(np.asarray(conv_w, dtype=F32))
    bias = np.ascontiguousarray(
        np.broadcast_to(np.asarray(conv_b, dtype=F32), (128, 128))
    )
    x0 = np.asarray(x_0, dtype=F32)
    xk = np.asarray(x_k, dtype=F32)
    wtr = np.ascontiguousarray(wt[:, 1024:])  # pairs 8..31, shared by all cores
    in_maps = []
    for r in range(NCORES):
        in0 = np.concatenate(
            [_pack_xk(xk[B * r:B * (r + 1)], dense), wt[:, :1024]], axis=1)
        in1 = np.concatenate(
            [_pack_x0(x0[B * r:B * (r + 1)], dense), bias], axis=1)
        in_maps.append({
            "in0_pack": np.ascontiguousarray(in0),
            "in1_pack": np.ascontiguousarray(in1),
            "wtr_pack": wtr,
        })
    return in_maps


VERSION = 20  # current best variant


def kernel(x_0, x_k, conv_w, conv_b):
    from concourse.bass_utils import run_bass_kernel_spmd

    nc = _build_program(VERSION)
    in_maps = pack_core_inputs(x_0, x_k, conv_w, conv_b, version=VERSION)
    res = run_bass_kernel_spmd(nc, in_maps, core_ids=list(range(NCORES)))
    out = np.empty((BS, NF, F), dtype=F32)
    for r in range(NCORES):
        _unpack_out(np.asarray(res.results[r]["out_pack"], dtype=F32), out, r)
    return out


# ---------------------------------------------------------------------------
# numpy model of the packed device program (for testing the packing logic)
# ---------------------------------------------------------------------------

def _numpy_model(x_0, x_k, conv_w, conv_b):
    out = np.empty((BS, NF, F), dtype=F32)
    in_maps = pack_core_inputs(x_0, x_k, conv_w, conv_b, version=2)
    for r in range(NCORES):
        m = in_maps[r]
        xk_s = m["in0_pack"][:, :1024]
        wt = np.concatenate([m["in0_pack"][:, 1024:], m["wtr_pack"]], axis=1)
        x0l = m["in1_pack"][:, :1024]
        bias = m["in1_pack"][:, 1024:1152]
        w2 = np.zeros((128, 1024), dtype=F32)
        for k in range(8):
            ps1 = np.zeros((128, 128), dtype=F32)
            for j in range(4):
                p = 4 * k + j
                ps1[32 * j:32 * (j + 1), :] = (
                    xk_s[:, 32 * p:32 * (p + 1)].T @ wt[:, 128 * p:128 * (p + 1)]
                )
            w2[:, 128 * k:128 * (k + 1)] = ps1
        # bounce: src partition (j,q,c2,h), free (k,n) -> dst [h,k,j,q,c2,n]
        srcA = w2.reshape(4, 2, 8, 2, 8, 128)          # [j,q,c2,h,k,n]
        w2b = srcA.transpose(3, 4, 0, 1, 2, 5)         # [h,k,j,q,c2,n]
        w2r = w2b.reshape(128, 8, 128).reshape(128, 1024)  # partition (h,k,j,q)
        out_pack = np.empty((128, 1024), dtype=F32)
        for c2 in range(8):
            out_pack[:, 128 * c2:128 * (c2 + 1)] = (
                x0l[:, 128 * c2:128 * (c2 + 1)].T @ w2r[:, 128 * c2:128 * (c2 + 1)]
                + bias
            )
        _unpack_out(out_pack, out, r)
    return out



# revision 27
# speedup vs baseline: 1.1591x; 1.0086x over previous
"""Trainium2 Bass kernel for the CIN-style layer:

    z   = einsum('btf,byf->bfty', x_0, x_k)            # pairwise outer products
    z   = z.reshape(bs, ts0, f, tsk)                   # flat reinterpretation
    out = einsum('btiy,nty->bni', z, conv_w) + conv_b  # strided conv reduction

Shapes: x_0 (32, 64, 256), x_k (32, 64, 256), conv_w (128, 64, 64),
conv_b (128,) -> out (32, 128, 256).

Math: with i = a*64 + m (a = i//64, m = i%64) and feature f = 4t + a the
reference reduces to a two-step factorization (~270 MFLOP vs 8.6 GFLOP naive):

    W2[b,n,t,a]      = sum_y x_k[b,y,4t+a] * conv_w[n,t,y]         (contract y)
    out[b,n,a*64+m]  = sum_t x_0[b,m,4t+a] * W2[b,n,t,a] + conv_b  (contract t)

Sharding: pure data parallel over batch, 4 samples per core, conv_w/conv_b
replicated (no collectives).

Shipped variant (VERSION=20 = v17 + step-2 PSUM split into two
bank-aligned tiles so bias-adds reading one half impose no false
whole-tile WAR on matmuls writing the other; all device data bf16,
rel err ~5e-3 vs the 2e-2 gate; fp32 baseline v3 was 36us on this box,
v20 is ~23.4us):
  step 1 (flipped vs v3): stationary = dense conv_w pair-tile
      [K=128 (tau,y), M=128 n], moving = block-diag xk tile [128, 32] ->
      PSUM W2T [n | 8*(2t+g)+c2] (c = 2*c2+g = 4b+a), 32 matmuls writing
      contiguous 32-col slices; per-group contiguous fp32->bf16 casts.
      This kills v3's 9us W2 DRAM bounce outright.
  transpose: 8 PE transposes, each reading one stride-8 single-free-dim
      slice [n | (2t+g)] of W2T (BIR requires one free dim on the
      stationary) -> PSUM [2t+g | n] bf16, copied to SBUF on vector.
  step 2: stationary = zero-padded interleaved x0 tile [K=128 (2t+g),
      M=128 (g',m)], moving = transposed W2 [128, 128 n] -> PSUM
      [64g'+m | (c2, n)]; bias fused into the PSUM->SBUF add; output
      shipped bf16 in eighths issued pairwise on both HWDGE rings and
      upcast to fp32 on the host.
  DMA: input is one [128, 6272] bf16 tensor (xk2 | wt2 | x0p | bias),
      six 256KB-ish chunks spread over the sync/scalar HWDGE rings and
      the gpsimd software-DGE ring; step-1 groups are emitted in chunk
      arrival order (1,3,0,2). 8 bf16 warmup matmuls cover the DMA window.

Known dead ends kept for reference: v13 (matmul is_transpose with a
non-identity moving operand is NOT a fused transposed-matmul - the moving
side must be a permutation matrix, results were garbage), v14 (split-K
step 2 with partition-offset transposes broke correctness), v16 (scalar
ACTIVATE copies + table preload measured slower - the preload was emitted
before scalar's DMA triggers and delayed the whole ring), v18 (splitting
the last weight chunk + its cast into halves measured slower), v19
(seeding bias into PSUM via a rank-1 matmul with accumulating step-2
matmuls measured slower and doubled rel err).

All layout work happens host-side in numpy; the device only runs
contiguous DMAs, matmuls, PE transposes, casts and copies.
"""

import numpy as np

BS, TS, F, NF = 32, 64, 256, 128
NCORES = 8
B = BS // NCORES  # 4 local batches per core

F32 = np.float32


# ---------------------------------------------------------------------------
# Host-side packing
# ---------------------------------------------------------------------------

def _pack_wt(conv_w: np.ndarray) -> np.ndarray:
    # WT[64q+y, 128p+n] = conv_w[n, 2p+q, y]
    wt = conv_w.transpose(1, 2, 0).reshape(32, 2, 64, NF)  # [p, q, y, n]
    wt = wt.transpose(1, 2, 0, 3)                          # [q, y, p, n]
    return np.ascontiguousarray(wt.reshape(128, 32 * NF), dtype=F32)


def _pack_xk(xk_shard: np.ndarray, dense=False) -> np.ndarray:
    # padded: XK[64q+y, 32p+16q'+c] = xk[b, y, 8p+4q+a] iff q'==q else 0
    # dense:  XKD[64q+y, 16p+c]     = xk[b, y, 8p+4q+a]          (c = 4b+a)
    xq = xk_shard.reshape(B, TS, 32, 2, 4)       # [b, y, p, q, a]
    src = xq.transpose(3, 1, 2, 0, 4)            # [q, y, p, b, a]
    if dense:
        return np.ascontiguousarray(src.reshape(128, 512))
    arr = np.zeros((2, TS, 32, 2, B, 4), dtype=F32)
    arr[0, :, :, 0] = src[0]
    arr[1, :, :, 1] = src[1]
    return arr.reshape(128, 32 * 32)


def _pack_x0(x0_shard: np.ndarray, dense=False) -> np.ndarray:
    # padded: X0L[64h+t, 128c2+64h'+m] = x0[b(c), m, 4t+a(c)] iff h'==h
    # dense:  X0D[64h+t, 64c2+m]       = x0[b(c), m, 4t+a(c)]   (c = 2*c2+h)
    xt = x0_shard.reshape(B, TS, TS, 4).transpose(0, 3, 2, 1)  # [b, a, t, m]
    flat = xt.reshape(16, TS, TS)                              # [c, t, m]
    if dense:
        arr = np.zeros((2, TS, 8, TS), dtype=F32)              # [h, t, c2, m]
        for h in (0, 1):
            arr[h] = flat[2 * np.arange(8) + h].transpose(1, 0, 2)
        return arr.reshape(128, 512)
    arr = np.zeros((2, TS, 8, 2, TS), dtype=F32)               # [h, t, c2, h', m]
    for h in (0, 1):
        arr[h, :, :, h, :] = flat[2 * np.arange(8) + h].transpose(1, 0, 2)
    return arr.reshape(128, 8 * 128)


def _unpack_out(out_pack: np.ndarray, out_full: np.ndarray, r: int) -> None:
    if VERSION == 13:
        # out_pack[n, 128c2+64g+m] = out[4r+b(c), n, a(c)*64+m], c = 2*c2+g
        o = out_pack.reshape(NF, 8, 2, TS)  # [n, c2, g, m]
        for c2 in range(8):
            for g in (0, 1):
                c = 2 * c2 + g
                b, a = divmod(c, 4)
                out_full[4 * r + b, :, a * TS:(a + 1) * TS] = o[:, c2, g, :]
        return
    # out_pack[64h+m, 128c2+n] = out[4r+b(c), n, a(c)*64+m], c = 2*c2+h
    o = out_pack.reshape(2, TS, 8, NF)  # [h, m, c2, n]
    for c2 in range(8):
        for h in (0, 1):
            c = 2 * c2 + h
            b, a = divmod(c, 4)
            out_full[4 * r + b, :, a * TS:(a + 1) * TS] = o[h, :, c2, :].T


# ---------------------------------------------------------------------------
# Device program
# ---------------------------------------------------------------------------

_prog_cache = {}


def _emit_body(nc, tc, pool, ps_pool, f32, in0_d, in1_d, wtr_d, out_d, w2b_d,
               version=2, stage="all"):
    # stage: "in" = input DMAs only, "s1" = through step-1 copies,
    #        "shuffle" = through the bounce, "all" = full kernel
    import concourse.mybir as mybir

    if version >= 3:
        # PE warm-up: ~3.4us of back-to-back matmuls on a zeroed bf16 tile
        # while the input DMAs stream in; gets the HAM clock gate to 2.4GHz
        # before step 1 starts.  No data deps -> scheduled first on PE.
        warm_s = pool.tile([128, 512], mybir.dt.bfloat16, tag="warm")
        nc.gpsimd.memset(warm_s[:], 0.0)
        ps_w = ps_pool.tile([128, 512], f32, tag="warm_ps")
        for _ in range(8):
            nc.tensor.matmul(ps_w[:, :], warm_s[:, 0:128], warm_s[:, :],
                             start=True, stop=True)

    dense = version in (6, 7, 8, 10)   # xk/x0 shipped dense, padded on-chip
    merged = version in (5, 7, 8, 9)   # single wtr DMA + single out DMA
    dual = version in (8, 9, 10)       # use both HWDGE rings (SP + ACT)
    eng2 = nc.scalar if dual else nc.sync

    nxk = 512 if dense else 1024   # xk cols in in0
    nx0 = 512 if dense else 1024   # x0 cols in in1
    in0_s = pool.tile([128, nxk + 1024], f32, tag="in0")
    nc.sync.dma_start(in0_s[:], in0_d.ap())
    wtr_s = []
    if merged:
        t_ = pool.tile([128, 3072], f32, tag="wtr")
        eng2.dma_start(t_[:], wtr_d.ap())
        wtr_s = [t_[:, 0:1024], t_[:, 1024:2048], t_[:, 2048:3072]]
    else:
        chunk_eng = [eng2, nc.sync, eng2]
        for chunk in range(3):
            t_ = pool.tile([128, 1024], f32, tag=f"wtr{chunk}")
            chunk_eng[chunk].dma_start(
                t_[:], wtr_d.ap()[:, 1024 * chunk:1024 * (chunk + 1)])
            wtr_s.append(t_[:])
    in1_s = pool.tile([128, nx0 + 128], f32, tag="in1")
    nc.sync.dma_start(in1_s[:], in1_d.ap())

    if dense:
        # zero-pad dense xk/x0 into block-diagonal lhsT layouts on-chip
        # (memsets + strided DVE copies are hidden under the DMA stream)
        xk_pad = pool.tile([128, 1024], f32, tag="xkpad")
        nc.gpsimd.memset(xk_pad[:], 0.0)
        x0_pad = pool.tile([128, 1024], f32, tag="x0pad")
        nc.gpsimd.memset(x0_pad[:], 0.0)
        for q in range(2):
            dst = xk_pad[64 * q:64 * (q + 1), :].rearrange(
                "p (a b) -> p a b", b=32)[:, :, 16 * q:16 * (q + 1)]
            src = in0_s[64 * q:64 * (q + 1), 0:512].rearrange(
                "p (a b) -> p a b", b=16)
            nc.vector.tensor_copy(dst, src)
        for h in range(2):
            dst = x0_pad[64 * h:64 * (h + 1), :].rearrange(
                "p (a b) -> p a b", b=128)[:, :, 64 * h:64 * (h + 1)]
            src = in1_s[64 * h:64 * (h + 1), 0:512].rearrange(
                "p (a b) -> p a b", b=64)
            nc.vector.tensor_copy(dst, src)
        xk_s = xk_pad[:, 0:1024]
        x0_s = x0_pad[:, 0:1024]
    else:
        xk_s = in0_s[:, 0:1024]
        x0_s = in1_s[:, 0:1024]
    bias_s = in1_s[:, nx0:nx0 + 128]

    def wt_cols(p):  # rhs tile [128, 128] for pair p
        if p < 8:
            return in0_s[:, nxk + 128 * p:nxk + 128 * (p + 1)]
        chunk, off = divmod(128 * (p - 8), 1024)
        return wtr_s[chunk][:, off:off + 128]

    if stage == "in":
        return

    # ---- step 1: W2 = xk . wT, contract y (K = 128 = (q, y)) ----
    # psum tile u holds passes 4u..4u+3 at col 128*(k%4)
    # ---- shuffle (q,c)-partitioned W2 -> t-partitioned via DRAM bounce ----
    # (a direct SBUF->SBUF partition-gather is impossible: the BIR verifier
    # rejects partition steps != 1 and the permutation needs >3 AP dims)
    w2_s = pool.tile([128, 1024], f32, tag="w2")
    w2r_s = pool.tile([128, 1024], f32, tag="w2r")

    def emit_pass(k, ps1):
        for j in range(4):
            p = 4 * k + j
            nc.tensor.matmul(
                ps1[32 * j:32 * (j + 1), 128 * (k % 4):128 * (k % 4 + 1)],
                xk_s[:, 32 * p:32 * (p + 1)],
                wt_cols(p),
                start=True,
                stop=True,
                tile_position=(0, 32 * j),
            )

    if version == 4:
        # per-pass copy + per-pass bounce-out (k fixed -> <=3 AP dims), so
        # all but the last bounce hides under step 1; readback in (h, k-half)
        # quarters, the first two of which also overlap step 1.
        # (measured WORSE on HW: per-DMA serialized overhead dominates)
        for u in range(2):
            ps1 = ps_pool.tile([128, 512], f32, tag="s1")
            for k in range(4 * u, 4 * u + 4):
                emit_pass(k, ps1)
                kk = 128 * (k % 4)
                nc.vector.tensor_copy(w2_s[:, 128 * k:128 * (k + 1)],
                                      ps1[:, kk:kk + 128])
                dstA = w2b_d.ap()[:, k].rearrange("h j q c2 n -> j q c2 h n")
                nc.sync.dma_start(dstA, w2_s[:, 128 * k:128 * (k + 1)])
            for h in range(2):
                kh = u
                dstB = w2r_s[64 * h + 32 * kh:64 * h + 32 * kh + 32, :]
                nc.sync.dma_start(dstB, w2b_d.ap()[h, 4 * kh:4 * (kh + 1)])
    else:
        for u in range(2):
            ps1 = ps_pool.tile([128, 512], f32, tag="s1")
            for k in range(4 * u, 4 * u + 4):
                emit_pass(k, ps1)
            nc.vector.tensor_copy(w2_s[:, 512 * u:512 * (u + 1)], ps1[:, :])
        if stage == "s1":
            return
        srcA = w2_s[:].rearrange("p (k n) -> p k n", k=8)
        dstA = w2b_d.ap().rearrange("h k j q c2 n -> j q c2 h k n")
        eng2.dma_start(dstA, srcA)
        dstB = w2r_s[:].rearrange("p (c2 n) -> p c2 n", c2=8)
        nc.sync.dma_start(dstB, w2b_d.ap())
        if stage == "shuffle":
            return

    # ---- step 2: out = x0 . W2, contract t (K = 128 = (h, t)) ----
    out_s = pool.tile([128, 1024], f32, tag="out")
    for u in range(2):
        ps2 = ps_pool.tile([128, 512], f32, tag="s2")
        for c2 in range(4 * u, 4 * u + 4):
            nc.tensor.matmul(
                ps2[:, 128 * (c2 % 4):128 * (c2 % 4 + 1)],
                x0_s[:, 128 * c2:128 * (c2 + 1)],
                w2r_s[:, 128 * c2:128 * (c2 + 1)],
                start=True,
                stop=True,
            )
        bias4 = bias_s.unsqueeze(1).broadcast_to([128, 4, 128])
        nc.vector.tensor_add(
            out_s[:, 512 * u:512 * (u + 1)].rearrange("p (f n) -> p f n", f=4),
            ps2[:, :].rearrange("p (f n) -> p f n", f=4),
            bias4,
        )
        if version >= 3 and not merged:
            (nc.sync if u == 0 else eng2).dma_start(
                out_d.ap()[:, 512 * u:512 * (u + 1)],
                out_s[:, 512 * u:512 * (u + 1)])
    if version == 2 or merged:
        eng2.dma_start(out_d.ap(), out_s[:])


def _pack_x0p(x0_shard: np.ndarray) -> np.ndarray:
    # x0p[2t+g, 128c2+64g'+m] = x0[b(c), m, 4t+a(c)] iff g==g', c = 2c2+g = 4b+a
    arr = np.zeros((TS, 2, 8, 2, TS), dtype=F32)  # [t, g, c2, g', m]
    for c in range(16):
        c2, g = divmod(c, 2)
        b, a = divmod(c, 4)
        arr[:, g, c2, g, :] = x0_shard[b, :, a::4].T  # [t, m]
    return arr.reshape(128, 1024)


def _emit_body_v11(nc, tc, pool, ps_pool, tp_pool, in_d, out_d, warm=8):
    """bf16 bounce-free pipeline.

    step 1 (flipped vs v3): stationary = dense conv_w pair-tile
      [K=128 (tau,y), M=128 n], moving = block-diag xk tile [128, 32 (tau',c)]
      -> PSUM W2T [n | 32p+16tau'+c] = [n | (t, c)], 32 matmuls, no bounce.
    transpose: 8 PE transposes of strided slices [n | (t, g)] (g = c parity,
      c = 2*c2+g) -> PSUM [2t+g | n] per c2, bf16.
    step 2: stationary = block-diag x0 tile [K=128 (2t+g), M=128 (g',m)],
      moving = transposed W2 [128, 128 n] -> PSUM [64g'+m | (c2, n)], the v3
      out_pack layout. Bias added in the PSUM->SBUF copy; output DMA'd bf16.
    """
    import concourse.mybir as mybir
    from concourse import masks

    bf16 = mybir.dt.bfloat16
    f32 = mybir.dt.float32

    # PE warm-up on a zeroed bf16 tile while input DMAs stream (p-state ramp)
    warm_s = pool.tile([128, 512], bf16, tag="warm")
    nc.gpsimd.memset(warm_s[:], 0.0)
    ps_w = ps_pool.tile([128, 512], f32, tag="warm_ps")
    for _ in range(warm):
        nc.tensor.matmul(ps_w[:, :], warm_s[:, 0:128], warm_s[:, :],
                         start=True, stop=True)

    ident = pool.tile([128, 128], bf16, tag="ident")
    masks.make_identity(nc, ident[:])

    # in_s cols: [0:1024 xk2 | 1024:5120 wt2 | 5120:6144 x0p | 6144:6272 bias]
    in_s = pool.tile([128, 6272], bf16, tag="in")
    nc.sync.dma_start(in_s[:, 0:1024], in_d.ap()[:, 0:1024])        # xk2
    nc.sync.dma_start(in_s[:, 1024:2048], in_d.ap()[:, 1024:2048])  # wt2 p0-7
    nc.sync.dma_start(in_s[:, 2048:3072], in_d.ap()[:, 2048:3072])  # wt2 p8-15
    nc.scalar.dma_start(in_s[:, 3072:4096], in_d.ap()[:, 3072:4096])  # p16-23
    nc.scalar.dma_start(in_s[:, 4096:5120], in_d.ap()[:, 4096:5120])  # p24-31
    nc.scalar.dma_start(in_s[:, 5120:6272], in_d.ap()[:, 5120:6272])  # x0p+bias

    xk2 = in_s[:, 0:1024]
    x0p = in_s[:, 5120:6144]
    bias_bf = in_s[:, 6144:6272]
    if v19:
        ones_s = pool.tile([1, 128], bf16, tag="ones")
        nc.gpsimd.memset(ones_s[:], 1.0)
    else:
        bias_s = pool.tile([128, 128], f32, tag="bias32")
        nc.gpsimd.tensor_copy(bias_s[:], bias_bf)  # upcast once for the DVE add

    # ---- step 1: W2T[n, 32p+16tau+c] -> psum [128, 1024] fp32 ----
    w2t_s = pool.tile([128, 1024], bf16, tag="w2t")
    if v21:
        # two bank-aligned halves, consecutive-emission groups alternate:
        # a cast reading one half no longer gates the next group's matmuls
        ps1a = ps_pool.tile([128, 512], f32, tag="s1a")
        ps1b = ps_pool.tile([128, 512], f32, tag="s1b")
        ps1_of = lambda g: (ps1a if g < 2 else ps1b, 256 * (g % 2))
    else:
        ps1 = ps_pool.tile([128, 1024], f32, tag="s1")
        ps1_of = lambda g: (ps1, 256 * g)
    # pair groups ordered to match DMA-chunk arrival (sync: 0-7, 8-15 after
    # xk2; scalar ring delivers 16-23 earliest)
    for grp in (2, 0, 3, 1):
        for p in range(8 * grp, 8 * grp + 8):
            nc.tensor.matmul(
                ps1[:, 32 * p:32 * (p + 1)],
                in_s[:, 1024 + 128 * p:1024 + 128 * (p + 1)],
                xk2[:, 32 * p:32 * (p + 1)],
                start=True, stop=True,
            )
        dst = w2t_s[:].rearrange("p (c t) -> p t c", t=64)[
            :, 16 * grp:16 * (grp + 1), :]
        src = ps1[:, 256 * grp:256 * (grp + 1)].rearrange(
            "p (t c) -> p t c", c=16)
        nc.vector.tensor_copy(dst, src)

    # ---- transpose + step 2, pipelined per c2 ----
    w2tt_s = pool.tile([128, 1024], bf16, tag="w2tt")
    out_s = pool.tile([128, 1024], bf16, tag="out")
    ps2 = ps_pool.tile([128, 1024], f32, tag="s2")
    for c2 in range(8):
        pst = tp_pool.tile([128, 128], bf16, tag="tp")
        nc.tensor.transpose(pst[:], w2t_s[:, 128 * c2:128 * (c2 + 1)],
                            ident[:])
        if c2 % 2:
            nc.scalar.copy(w2tt_s[:, 128 * c2:128 * (c2 + 1)], pst[:])
        else:
            nc.vector.tensor_copy(w2tt_s[:, 128 * c2:128 * (c2 + 1)], pst[:])
        nc.tensor.matmul(
            ps2[:, 128 * c2:128 * (c2 + 1)],
            x0p[:, 128 * c2:128 * (c2 + 1)],
            w2tt_s[:, 128 * c2:128 * (c2 + 1)],
            start=True, stop=True,
        )
        if c2 == 3 or c2 == 7:
            u = c2 // 4
            bias4 = bias_s[:].unsqueeze(1).broadcast_to([128, 4, 128])
            nc.vector.tensor_add(
                out_s[:, 512 * u:512 * (u + 1)].rearrange(
                    "p (f n) -> p f n", f=4),
                ps2[:, 512 * u:512 * (u + 1)].rearrange(
                    "p (f n) -> p f n", f=4),
                bias4,
            )
            (nc.sync if u == 0 else nc.scalar).dma_start(
                out_d.ap()[:, 512 * u:512 * (u + 1)],
                out_s[:, 512 * u:512 * (u + 1)])


def _emit_body_v12(nc, tc, pool, ps_pool, tp_pool, in_d, out_d, warm=8):
    """v11 + reordered W2T columns (8*(2t+g)+c2) so each c2 transpose input
    is one stride-8 free dim; single contiguous fp32->bf16 cast; vector-only
    PSUM copies (no ACT table load); gpsimd as third input DMA ring."""
    import concourse.mybir as mybir
    from concourse import masks

    bf16 = mybir.dt.bfloat16
    f32 = mybir.dt.float32

    warm_s = pool.tile([128, 512], bf16, tag="warm")
    nc.gpsimd.memset(warm_s[:], 0.0)
    ps_w = ps_pool.tile([128, 512], f32, tag="warm_ps")
    for _ in range(warm):
        nc.tensor.matmul(ps_w[:, :], warm_s[:, 0:128], warm_s[:, :],
                         start=True, stop=True)

    ident = pool.tile([128, 128], bf16, tag="ident")
    masks.make_identity(nc, ident[:])

    # in_s cols: [0:1024 xk2 | 1024:5120 wt2 | 5120:6144 x0p | 6144:6272 bias]
    in_s = pool.tile([128, 6272], bf16, tag="in")
    nc.sync.dma_start(in_s[:, 0:1024], in_d.ap()[:, 0:1024])          # xk2
    nc.sync.dma_start(in_s[:, 1024:2048], in_d.ap()[:, 1024:2048])    # p0-7
    nc.sync.dma_start(in_s[:, 2048:3072], in_d.ap()[:, 2048:3072])    # p8-15
    nc.scalar.dma_start(in_s[:, 3072:4096], in_d.ap()[:, 3072:4096])  # p16-23
    nc.scalar.dma_start(in_s[:, 4096:5120], in_d.ap()[:, 4096:5120])  # p24-31
    nc.gpsimd.dma_start(in_s[:, 5120:6272], in_d.ap()[:, 5120:6272])  # x0p+b

    xk2 = in_s[:, 0:1024]
    x0p = in_s[:, 5120:6144]
    bias_bf = in_s[:, 6144:6272]
    if v19:
        ones_s = pool.tile([1, 128], bf16, tag="ones")
        nc.gpsimd.memset(ones_s[:], 1.0)
    else:
        bias_s = pool.tile([128, 128], f32, tag="bias32")
        nc.gpsimd.tensor_copy(bias_s[:], bias_bf)

    # ---- step 1: psum cols 32p+16tau+(8g+c2), contiguous writes ----
    w2t_s = pool.tile([128, 1024], bf16, tag="w2t")
    if v21:
        # two bank-aligned halves, consecutive-emission groups alternate:
        # a cast reading one half no longer gates the next group's matmuls
        ps1a = ps_pool.tile([128, 512], f32, tag="s1a")
        ps1b = ps_pool.tile([128, 512], f32, tag="s1b")
        ps1_of = lambda g: (ps1a if g < 2 else ps1b, 256 * (g % 2))
    else:
        ps1 = ps_pool.tile([128, 1024], f32, tag="s1")
        ps1_of = lambda g: (ps1, 256 * g)
    for grp in (2, 0, 3, 1):
        for p in range(8 * grp, 8 * grp + 8):
            nc.tensor.matmul(
                ps1[:, 32 * p:32 * (p + 1)],
                in_s[:, 1024 + 128 * p:1024 + 128 * (p + 1)],
                xk2[:, 32 * p:32 * (p + 1)],
                start=True, stop=True,
            )
    nc.vector.tensor_copy(w2t_s[:], ps1[:])  # one contiguous cast

    # ---- transpose (stride-8 read) + step 2, pipelined per c2 ----
    w2tt_s = pool.tile([128, 1024], bf16, tag="w2tt")
    out_s = pool.tile([128, 1024], bf16, tag="out")
    ps2 = ps_pool.tile([128, 1024], bf16, tag="s2")
    w2t_v = w2t_s[:].rearrange("p (tg c2) -> p tg c2", c2=8)
    for c2 in range(8):
        pst = tp_pool.tile([128, 128], bf16, tag="tp")
        nc.tensor.transpose(pst[:], w2t_v[:, :, c2:c2 + 1], ident[:])
        if v16 and c2 % 2 == 0:
            nc.scalar.copy(w2tt_s[:, 128 * c2:128 * (c2 + 1)], pst[:])
        else:
            nc.vector.tensor_copy(w2tt_s[:, 128 * c2:128 * (c2 + 1)], pst[:])
        nc.tensor.matmul(
            ps2[:, 128 * c2:128 * (c2 + 1)],
            x0p[:, 128 * c2:128 * (c2 + 1)],
            w2tt_s[:, 128 * c2:128 * (c2 + 1)],
            start=True, stop=True,
        )
        if c2 == 3 or c2 == 7:
            u = c2 // 4
            bias4 = bias_s[:].unsqueeze(1).broadcast_to([128, 4, 128])
            nc.vector.tensor_add(
                out_s[:, 512 * u:512 * (u + 1)].rearrange(
                    "p (f n) -> p f n", f=4),
                ps2[:, 512 * u:512 * (u + 1)].rearrange(
                    "p (f n) -> p f n", f=4),
                bias4,
            )
            (nc.sync if u == 0 else nc.scalar).dma_start(
                out_d.ap()[:, 512 * u:512 * (u + 1)],
                out_s[:, 512 * u:512 * (u + 1)])


def _emit_body_v13(nc, tc, pool, ps_pool, in_d, out_d, warm=8):
    """v12 + transpose fused into step 2: matmul(is_transpose=True) loads the
    W2T stride-8 slice transposed as stationary and streams x0p as moving ->
    out [n | (c2, g, m)] with no PE transposes, no tp psum tiles, no copies.
    Casts are per-group (contiguous). Bias is per-partition (n)."""
    import concourse.mybir as mybir

    bf16 = mybir.dt.bfloat16
    f32 = mybir.dt.float32

    warm_s = pool.tile([128, 512], bf16, tag="warm")
    nc.gpsimd.memset(warm_s[:], 0.0)
    ps_w = ps_pool.tile([128, 512], f32, tag="warm_ps")
    for _ in range(warm):
        nc.tensor.matmul(ps_w[:, :], warm_s[:, 0:128], warm_s[:, :],
                         start=True, stop=True)

    # in_s cols: [0:1024 xk2 | 1024:5120 wt2 | 5120:6144 x0p | 6144:6272 bias]
    in_s = pool.tile([128, 6272], bf16, tag="in")
    nc.sync.dma_start(in_s[:, 0:1024], in_d.ap()[:, 0:1024])          # xk2
    nc.sync.dma_start(in_s[:, 1024:2048], in_d.ap()[:, 1024:2048])    # p0-7
    nc.scalar.dma_start(in_s[:, 2048:3072], in_d.ap()[:, 2048:3072])  # p8-15
    nc.scalar.dma_start(in_s[:, 3072:4096], in_d.ap()[:, 3072:4096])  # p16-23
    nc.gpsimd.dma_start(in_s[:, 4096:5120], in_d.ap()[:, 4096:5120])  # p24-31
    nc.gpsimd.dma_start(in_s[:, 5120:6272], in_d.ap()[:, 5120:6272])  # x0p+b

    xk2 = in_s[:, 0:1024]
    x0p = in_s[:, 5120:6144]
    bias_bf = in_s[:, 6144:6272]   # [n, j] = conv_b[n] (row-indexed)
    bias_s = pool.tile([128, 128], f32, tag="bias32")
    nc.gpsimd.tensor_copy(bias_s[:], bias_bf)

    # ---- step 1 + per-group contiguous casts ----
    w2t_s = pool.tile([128, 1024], bf16, tag="w2t")
    if v21:
        # two bank-aligned halves, consecutive-emission groups alternate:
        # a cast reading one half no longer gates the next group's matmuls
        ps1a = ps_pool.tile([128, 512], f32, tag="s1a")
        ps1b = ps_pool.tile([128, 512], f32, tag="s1b")
        ps1_of = lambda g: (ps1a if g < 2 else ps1b, 256 * (g % 2))
    else:
        ps1 = ps_pool.tile([128, 1024], f32, tag="s1")
        ps1_of = lambda g: (ps1, 256 * g)
    # (start_pair, n_pairs) subgroups in chunk-arrival order; each is
    # followed by a contiguous cast of just its psum columns
    subgroups = ([(8, 8), (24, 8), (16, 4), (0, 8), (20, 4)] if v18
                 else [(8, 8), (24, 8), (0, 8), (16, 8)])
    for p0, np_ in subgroups:
        pt1, o1 = ps1_of(p0 // 8)
        for p in range(p0, p0 + np_):
            o = o1 + 32 * (p - 8 * (p0 // 8))
            nc.tensor.matmul(
                pt1[:, o:o + 32],
                in_s[:, 1024 + 128 * p:1024 + 128 * (p + 1)],
                xk2[:, 32 * p:32 * (p + 1)],
                start=True, stop=True,
            )
        nc.vector.tensor_copy(w2t_s[:, 32 * p0:32 * (p0 + np_)],
                              pt1[:, o1:o1 + 32 * np_])

    # ---- step 2: transposed-load W2T slice (stride 8) x moving x0p ----
    out_s = pool.tile([128, 1024], bf16, tag="out")
    ps2 = ps_pool.tile([128, 1024], bf16, tag="s2")
    w2t_v = w2t_s[:].rearrange("p (tg c2) -> p tg c2", c2=8)
    for c2 in range(8):
        nc.tensor.matmul(
            ps2[:, 128 * c2:128 * (c2 + 1)],
            w2t_v[:, :, c2:c2 + 1],
            x0p[:, 128 * c2:128 * (c2 + 1)],
            is_transpose=True,
            start=True, stop=True,
        )
        if c2 == 3 or c2 == 7:
            u = c2 // 4
            bias_b = bias_s[:, 0:1].broadcast_to([128, 512])
            nc.vector.tensor_add(
                out_s[:, 512 * u:512 * (u + 1)],
                ps2[:, 512 * u:512 * (u + 1)],
                bias_b,
            )
            (nc.sync if u == 0 else nc.scalar).dma_start(
                out_d.ap()[:, 512 * u:512 * (u + 1)],
                out_s[:, 512 * u:512 * (u + 1)])


def _emit_body_v14(nc, tc, pool, ps_pool, tp_pool, in_d, out_d, warm=8):
    """v13 DMA/cast structure + v12-style PE transposes, split-K step 2:
    each c2 accumulates two t-halves in PSUM, so half-A transposes+matmuls
    overlap the tail of the weight stream. Quarter-granularity output."""
    import concourse.mybir as mybir
    from concourse import masks

    bf16 = mybir.dt.bfloat16
    f32 = mybir.dt.float32

    warm_s = pool.tile([128, 512], bf16, tag="warm")
    nc.gpsimd.memset(warm_s[:], 0.0)
    ps_w = ps_pool.tile([128, 512], f32, tag="warm_ps")
    for _ in range(warm):
        nc.tensor.matmul(ps_w[:, :], warm_s[:, 0:128], warm_s[:, :],
                         start=True, stop=True)

    ident = pool.tile([128, 128], bf16, tag="ident")
    masks.make_identity(nc, ident[:])

    # in_s cols: [0:1024 xk2 | 1024:5120 wt2 | 5120:6144 x0p | 6144:6272 bias]
    in_s = pool.tile([128, 6272], bf16, tag="in")
    ap = in_d.ap()
    nc.sync.dma_start(in_s[:, 0:1024], ap[:, 0:1024])                # xk2
    nc.scalar.dma_start(in_s[:, 1024:2048], ap[:, 1024:2048])        # p0-7
    nc.gpsimd.dma_start(in_s[0:64, 5120:6144], ap[0:64, 5120:6144])  # x0p top
    nc.gpsimd.dma_start(in_s[:, 2048:3072], ap[:, 2048:3072])        # p8-15
    nc.sync.dma_start(in_s[:, 3072:4096], ap[:, 3072:4096])          # p16-23
    nc.scalar.dma_start(in_s[:, 4096:5120], ap[:, 4096:5120])        # p24-31
    nc.sync.dma_start(in_s[64:128, 5120:6144], ap[64:128, 5120:6144])  # x0p bot
    nc.scalar.dma_start(in_s[:, 6144:6272], ap[:, 6144:6272])        # bias

    xk2 = in_s[:, 0:1024]
    x0p = in_s[:, 5120:6144]
    bias_bf = in_s[:, 6144:6272]
    if v19:
        ones_s = pool.tile([1, 128], bf16, tag="ones")
        nc.gpsimd.memset(ones_s[:], 1.0)
    else:
        bias_s = pool.tile([128, 128], f32, tag="bias32")
        nc.gpsimd.tensor_copy(bias_s[:], bias_bf)

    w2t_s = pool.tile([128, 1024], bf16, tag="w2t")
    if v21:
        # two bank-aligned halves, consecutive-emission groups alternate:
        # a cast reading one half no longer gates the next group's matmuls
        ps1a = ps_pool.tile([128, 512], f32, tag="s1a")
        ps1b = ps_pool.tile([128, 512], f32, tag="s1b")
        ps1_of = lambda g: (ps1a if g < 2 else ps1b, 256 * (g % 2))
    else:
        ps1 = ps_pool.tile([128, 1024], f32, tag="s1")
        ps1_of = lambda g: (ps1, 256 * g)
    w2tt_s = pool.tile([128, 1024], bf16, tag="w2tt")
    out_s = pool.tile([128, 1024], bf16, tag="out")
    if v20:
        # two bank-aligned tiles: bias-adds reading the low half no longer
        # impose a false whole-tile WAR on matmuls writing the high half
        ps2a = ps_pool.tile([128, 512], f32, tag="s2a")
        ps2b = ps_pool.tile([128, 512], f32, tag="s2b")
        ps2_of = lambda c2: (ps2a if c2 < 4 else ps2b, (c2 % 4) * 128)
    else:
        ps2 = ps_pool.tile([128, 1024], f32, tag="s2")
        ps2_of = lambda c2: (ps2, c2 * 128)
    w2t_v = w2t_s[:].rearrange("p (tg c2) -> p tg c2", c2=8)

    def s1_group(grp):
        for p in range(8 * grp, 8 * grp + 8):
            nc.tensor.matmul(
                ps1[:, 32 * p:32 * (p + 1)],
                in_s[:, 1024 + 128 * p:1024 + 128 * (p + 1)],
                xk2[:, 32 * p:32 * (p + 1)],
                start=True, stop=True,
            )
        nc.vector.tensor_copy(w2t_s[:, 256 * grp:256 * (grp + 1)],
                              ps1[:, 256 * grp:256 * (grp + 1)])

    def s2_half(h):
        lo, hi = 64 * h, 64 * (h + 1)
        for c2 in range(8):
            pst = tp_pool.tile([128, 128], bf16, tag="tp")
            nc.tensor.transpose(pst[lo:hi, :], w2t_v[:, lo:hi, c2:c2 + 1],
                                ident[:])
            nc.vector.tensor_copy(w2tt_s[lo:hi, 128 * c2:128 * (c2 + 1)],
                                  pst[lo:hi, :])
            nc.tensor.matmul(
                ps2[:, 128 * c2:128 * (c2 + 1)],
                x0p[lo:hi, 128 * c2:128 * (c2 + 1)],
                w2tt_s[lo:hi, 128 * c2:128 * (c2 + 1)],
                start=(h == 0), stop=(h == 1),
            )
            if h == 1 and c2 % 2 == 1:
                u = c2 // 2
                bias4 = bias_s[:].unsqueeze(1).broadcast_to([128, 2, 128])
                nc.vector.tensor_add(
                    out_s[:, 256 * u:256 * (u + 1)].rearrange(
                        "p (f n) -> p f n", f=2),
                    ps2[:, 256 * u:256 * (u + 1)].rearrange(
                        "p (f n) -> p f n", f=2),
                    bias4,
                )
                (nc.sync if u % 2 == 0 else nc.scalar).dma_start(
                    out_d.ap()[:, 256 * u:256 * (u + 1)],
                    out_s[:, 256 * u:256 * (u + 1)])

    s1_group(0)
    s1_group(1)
    s2_half(0)
    s1_group(2)
    s1_group(3)
    s2_half(1)


def _emit_body_v15(nc, tc, pool, ps_pool, tp_pool, in_d, out_d, warm=8,
                   v16=False, v17=False, v18=False, v19=False, v20=False,
                   v21=False):
    """v13 DMA/cast structure + v12 transpose step 2 + quarter outputs.
    v16: scalar ACT-table preloaded during warmup, PSUM->SBUF copies
    alternate vector/scalar, deeper transpose pool."""
    import concourse.mybir as mybir
    from concourse import masks

    bf16 = mybir.dt.bfloat16
    f32 = mybir.dt.float32

    warm_s = pool.tile([128, 512], bf16, tag="warm")
    nc.gpsimd.memset(warm_s[:], 0.0)
    ps_w = ps_pool.tile([128, 512], f32, tag="warm_ps")
    for _ in range(warm):
        nc.tensor.matmul(ps_w[:, :], warm_s[:, 0:128], warm_s[:, :],
                         start=True, stop=True)

    ident = pool.tile([128, 128], bf16, tag="ident")
    masks.make_identity(nc, ident[:])
    if v16:
        # touch ACTIVATE during warmup so the 1.3us table load is off the
        # critical path when scalar copies run in the transpose phase
        nc.scalar.copy(warm_s[0:1, 0:1], warm_s[0:1, 1:2])

    in_s = pool.tile([128, 6272], bf16, tag="in")
    ap = in_d.ap()
    nc.sync.dma_start(in_s[:, 0:1024], ap[:, 0:1024])          # xk2
    nc.sync.dma_start(in_s[:, 1024:2048], ap[:, 1024:2048])    # p0-7
    nc.scalar.dma_start(in_s[:, 2048:3072], ap[:, 2048:3072])  # p8-15
    if v18:
        nc.scalar.dma_start(in_s[:, 3072:3584], ap[:, 3072:3584])  # p16-19
        nc.scalar.dma_start(in_s[:, 3584:4096], ap[:, 3584:4096])  # p20-23
    else:
        nc.scalar.dma_start(in_s[:, 3072:4096], ap[:, 3072:4096])  # p16-23
    nc.gpsimd.dma_start(in_s[:, 4096:5120], ap[:, 4096:5120])  # p24-31
    nc.gpsimd.dma_start(in_s[:, 5120:6272], ap[:, 5120:6272])  # x0p+bias

    xk2 = in_s[:, 0:1024]
    x0p = in_s[:, 5120:6144]
    bias_bf = in_s[:, 6144:6272]
    if v19:
        ones_s = pool.tile([1, 128], bf16, tag="ones")
        nc.gpsimd.memset(ones_s[:], 1.0)
    else:
        bias_s = pool.tile([128, 128], f32, tag="bias32")
        nc.gpsimd.tensor_copy(bias_s[:], bias_bf)

    w2t_s = pool.tile([128, 1024], bf16, tag="w2t")
    if v21:
        # two bank-aligned halves, consecutive-emission groups alternate:
        # a cast reading one half no longer gates the next group's matmuls
        ps1a = ps_pool.tile([128, 512], f32, tag="s1a")
        ps1b = ps_pool.tile([128, 512], f32, tag="s1b")
        ps1_of = lambda g: (ps1a if g < 2 else ps1b, 256 * (g % 2))
    else:
        ps1 = ps_pool.tile([128, 1024], f32, tag="s1")
        ps1_of = lambda g: (ps1, 256 * g)
    # (start_pair, n_pairs) subgroups in chunk-arrival order; each is
    # followed by a contiguous cast of just its psum columns
    subgroups = ([(8, 8), (24, 8), (16, 4), (0, 8), (20, 4)] if v18
                 else [(8, 8), (24, 8), (0, 8), (16, 8)])
    for p0, np_ in subgroups:
        pt1, o1 = ps1_of(p0 // 8)
        for p in range(p0, p0 + np_):
            o = o1 + 32 * (p - 8 * (p0 // 8))
            nc.tensor.matmul(
                pt1[:, o:o + 32],
                in_s[:, 1024 + 128 * p:1024 + 128 * (p + 1)],
                xk2[:, 32 * p:32 * (p + 1)],
                start=True, stop=True,
            )
        nc.vector.tensor_copy(w2t_s[:, 32 * p0:32 * (p0 + np_)],
                              pt1[:, o1:o1 + 32 * np_])

    w2tt_s = pool.tile([128, 1024], bf16, tag="w2tt")
    out_s = pool.tile([128, 1024], bf16, tag="out")
    if v20:
        # two bank-aligned tiles: bias-adds reading the low half no longer
        # impose a false whole-tile WAR on matmuls writing the high half
        ps2a = ps_pool.tile([128, 512], f32, tag="s2a")
        ps2b = ps_pool.tile([128, 512], f32, tag="s2b")
        ps2_of = lambda c2: (ps2a if c2 < 4 else ps2b, (c2 % 4) * 128)
    else:
        ps2 = ps_pool.tile([128, 1024], f32, tag="s2")
        ps2_of = lambda c2: (ps2, c2 * 128)
    w2t_v = w2t_s[:].rearrange("p (tg c2) -> p tg c2", c2=8)
    if v19:
        # seed every ps2 block with bias via rank-1 matmul (ones x bias row)
        # in the idle PE window; step-2 matmuls then accumulate on top
        for c2 in range(8):
            nc.tensor.matmul(
                ps2[:, 128 * c2:128 * (c2 + 1)],
                ones_s[:, 0:128],
                in_s[0:1, 6144:6272],
                start=True, stop=False,
            )
    for c2 in range(8):
        pst = tp_pool.tile([128, 128], bf16, tag="tp")
        nc.tensor.transpose(pst[:], w2t_v[:, :, c2:c2 + 1], ident[:])
        if v16 and c2 % 2 == 0:
            nc.scalar.copy(w2tt_s[:, 128 * c2:128 * (c2 + 1)], pst[:])
        else:
            nc.vector.tensor_copy(w2tt_s[:, 128 * c2:128 * (c2 + 1)], pst[:])
        pt, off = ps2_of(c2)
        nc.tensor.matmul(
            pt[:, off:off + 128],
            x0p[:, 128 * c2:128 * (c2 + 1)],
            w2tt_s[:, 128 * c2:128 * (c2 + 1)],
            start=not v19, stop=True,
        )
        if c2 % 2 == 1:
            u = c2 // 2
            if v19:
                nc.vector.tensor_copy(out_s[:, 256 * u:256 * (u + 1)],
                                      pt[:, off - 128:off + 128])
            else:
                bias4 = bias_s[:].unsqueeze(1).broadcast_to([128, 2, 128])
                nc.vector.tensor_add(
                    out_s[:, 256 * u:256 * (u + 1)].rearrange(
                        "p (f n) -> p f n", f=2),
                    pt[:, off - 128:off + 128].rearrange(
                        "p (f n) -> p f n", f=2),
                    bias4,
                )
            (nc.sync if u % 2 == 0 else nc.scalar).dma_start(
                out_d.ap()[:, 256 * u:256 * (u + 1)],
                out_s[:, 256 * u:256 * (u + 1)])


def _build_program(version=None):
    if version is None:
        version = VERSION
    if version in _prog_cache:
        return _prog_cache[version]

    from contextlib import ExitStack

    import concourse.bacc as bacc
    import concourse.mybir as mybir
    import concourse.tile as tile

    f32 = mybir.dt.float32
    nc = bacc.Bacc("TRN2", target_bir_lowering=False, debug=False)

    if version >= 11:
        bf16 = mybir.dt.bfloat16
        in_d = nc.dram_tensor("in_pack", [128, 6272], bf16, kind="ExternalInput")
        out_d = nc.dram_tensor("out_pack", [128, 1024], bf16,
                               kind="ExternalOutput")
        with tile.TileContext(nc) as tc, ExitStack() as ctx:
            pool = ctx.enter_context(tc.tile_pool(name="io", bufs=1))
            ps_pool = ctx.enter_context(
                tc.tile_pool(name="ps", bufs=1, space="PSUM"))
            tp_pool = ctx.enter_context(
                tc.tile_pool(name="tp", bufs=2 if version == 15 else 3,
                             space="PSUM"))
            if version >= 15:
                _emit_body_v15(nc, tc, pool, ps_pool, tp_pool, in_d, out_d,
                               v16=(version == 16),
                               v17=(version >= 17),
                               v18=(version == 18),
                               v19=(version == 19),
                               v20=(version >= 20),
                               v21=(version >= 21))
            elif version >= 14:
                _emit_body_v14(nc, tc, pool, ps_pool, tp_pool, in_d, out_d)
            elif version >= 13:
                _emit_body_v13(nc, tc, pool, ps_pool, in_d, out_d)
            elif version >= 12:
                _emit_body_v12(nc, tc, pool, ps_pool, tp_pool, in_d, out_d)
            else:
                _emit_body_v11(nc, tc, pool, ps_pool, tp_pool, in_d, out_d)
        nc.compile()
        _prog_cache[version] = nc
        return nc

    dense = version in (6, 7, 8, 10)
    nx = 512 if dense else 1024
    # in0 = [xk_pack | wt chunk0 (1024)], in1 = [x0_pack | bias (128)]
    in0_d = nc.dram_tensor("in0_pack", [128, nx + 1024], f32, kind="ExternalInput")
    in1_d = nc.dram_tensor("in1_pack", [128, nx + 128], f32, kind="ExternalInput")
    wtr_d = nc.dram_tensor("wtr_pack", [128, 3072], f32, kind="ExternalInput")
    out_d = nc.dram_tensor("out_pack", [128, 1024], f32, kind="ExternalOutput")
    # bounce layout [h, k, j, q, c2, n]
    w2b_d = nc.dram_tensor("w2_bounce", [2, 8, 4, 2, 8, 128], f32)

    with tile.TileContext(nc) as tc, ExitStack() as ctx:
        pool = ctx.enter_context(tc.tile_pool(name="io", bufs=1))
        ps_pool = ctx.enter_context(tc.tile_pool(name="ps", bufs=2, space="PSUM"))
        _emit_body(nc, tc, pool, ps_pool, f32, in0_d, in1_d, wtr_d, out_d, w2b_d,
                   version=version)

    nc.compile()
    _prog_cache[version] = nc
    return nc


def pack_core_inputs(x_0, x_k, conv_w, conv_b, version=None):
    """Returns (in_maps list of 8 dicts) for run_bass_kernel_spmd."""
    if version is None:
        version = VERSION
    if version >= 11:
        import ml_dtypes
        BF = ml_dtypes.bfloat16
        wt = _pack_wt(np.asarray(conv_w, dtype=F32))
        bias = np.broadcast_to(np.asarray(conv_b, dtype=F32), (128, 128))
        x0 = np.asarray(x_0, dtype=F32)
        xk = np.asarray(x_k, dtype=F32)
        in_maps = []
        for r in range(NCORES):
            xk2 = _pack_xk(xk[B * r:B * (r + 1)])
            if version >= 12:
                # block col order (tau, g, c2): newpos 8g+c2 <- c = 2c2+g
                cperm = np.array([2 * (i % 8) + i // 8 for i in range(16)])
                xk2 = xk2.reshape(128, 32, 2, 16)[:, :, :, cperm].reshape(
                    128, 1024)
                x0l = _pack_x0p(x0[B * r:B * (r + 1)])
            else:
                x0l = _pack_x0(x0[B * r:B * (r + 1)])
            b_blk = (np.broadcast_to(
                np.asarray(conv_b, dtype=F32)[:, None], (128, 128))
                if version == 13 else bias)
            in_pack = np.concatenate([xk2, wt, x0l, b_blk], axis=1)
            in_maps.append({"in_pack": np.ascontiguousarray(
                in_pack.astype(BF))})
        return in_maps
    dense = version in (6, 7, 8, 10)
    wt = _pack_wt(np.asarray(conv_w, dtype=F32))
    bias = np.ascontiguousarray(
        np.broadcast_to(np.asarray(conv_b, dtype=F32), (128, 128))
    )
    x0 = np.asarray(x_0, dtype=F32)
    xk = np.asarray(x_k, dtype=F32)
    wtr = np.ascontiguousarray(wt[:, 1024:])  # pairs 8..31, shared by all cores
    in_maps = []
    for r in range(NCORES):
        in0 = np.concatenate(
            [_pack_xk(xk[B * r:B * (r + 1)], dense), wt[:, :1024]], axis=1)
        in1 = np.concatenate(
            [_pack_x0(x0[B * r:B * (r + 1)], dense), bias], axis=1)
        in_maps.append({
            "in0_pack": np.ascontiguousarray(in0),
            "in1_pack": np.ascontiguousarray(in1),
            "wtr_pack": wtr,
        })
    return in_maps


VERSION = 21  # current best variant


def kernel(x_0, x_k, conv_w, conv_b):
    from concourse.bass_utils import run_bass_kernel_spmd

    nc = _build_program(VERSION)
    in_maps = pack_core_inputs(x_0, x_k, conv_w, conv_b, version=VERSION)
    res = run_bass_kernel_spmd(nc, in_maps, core_ids=list(range(NCORES)))
    out = np.empty((BS, NF, F), dtype=F32)
    for r in range(NCORES):
        _unpack_out(np.asarray(res.results[r]["out_pack"], dtype=F32), out, r)
    return out


# ---------------------------------------------------------------------------
# numpy model of the packed device program (for testing the packing logic)
# ---------------------------------------------------------------------------

def _numpy_model(x_0, x_k, conv_w, conv_b):
    out = np.empty((BS, NF, F), dtype=F32)
    in_maps = pack_core_inputs(x_0, x_k, conv_w, conv_b, version=2)
    for r in range(NCORES):
        m = in_maps[r]
        xk_s = m["in0_pack"][:, :1024]
        wt = np.concatenate([m["in0_pack"][:, 1024:], m["wtr_pack"]], axis=1)
        x0l = m["in1_pack"][:, :1024]
        bias = m["in1_pack"][:, 1024:1152]
        w2 = np.zeros((128, 1024), dtype=F32)
        for k in range(8):
            ps1 = np.zeros((128, 128), dtype=F32)
            for j in range(4):
                p = 4 * k + j
                ps1[32 * j:32 * (j + 1), :] = (
                    xk_s[:, 32 * p:32 * (p + 1)].T @ wt[:, 128 * p:128 * (p + 1)]
                )
            w2[:, 128 * k:128 * (k + 1)] = ps1
        # bounce: src partition (j,q,c2,h), free (k,n) -> dst [h,k,j,q,c2,n]
        srcA = w2.reshape(4, 2, 8, 2, 8, 128)          # [j,q,c2,h,k,n]
        w2b = srcA.transpose(3, 4, 0, 1, 2, 5)         # [h,k,j,q,c2,n]
        w2r = w2b.reshape(128, 8, 128).reshape(128, 1024)  # partition (h,k,j,q)
        out_pack = np.empty((128, 1024), dtype=F32)
        for c2 in range(8):
            out_pack[:, 128 * c2:128 * (c2 + 1)] = (
                x0l[:, 128 * c2:128 * (c2 + 1)].T @ w2r[:, 128 * c2:128 * (c2 + 1)]
                + bias
            )
        _unpack_out(out_pack, out, r)
    return out

